# revision 10
# baseline (speedup 1.0000x reference)
# MoE layer (8 experts, top-2 routing) on 8 trn2 NeuronCores.
# Sharding: token-parallel. Each core processes 512 of the 4096 tokens:
# full router + all 8 experts (dense eval, matching the reference) on its
# token slice. No collectives; host concatenates the per-core outputs.
#
# Layouts: activations are feature-major [feat_part, tok_free] through the
# MLPs (contraction dim always on partitions -> no transposes); the last
# expert matmul (fc2) swaps lhsT/rhs so its output lands token-major
# [tok_part, feat_free], where layernorm + the prob-weighted combine only
# need per-partition scalars. The input-independent attr projection and the
# ap-half of gate_w1 are folded into host-precomputed biases.
import numpy as np
import ml_dtypes
from contextlib import ExitStack

import concourse.bass as bass
import concourse.bacc as bacc
import concourse.mybir as mybir
import concourse.tile as tile
from concourse.bass import ts
from concourse.bass_utils import run_bass_kernel_spmd

F32 = mybir.dt.float32
BF16 = mybir.dt.bfloat16
AF = mybir.ActivationFunctionType
ALU = mybir.AluOpType
AXX = mybir.AxisListType.X

H, E, TD, NT, TOPK, I = 1024, 8, 64, 4, 2, 2048
A = TD * NT
B, S = 4, 1024
NTOK = B * S
NCORE = 8
T = NTOK // NCORE          # tokens per core
P = 128
TCH = T // P               # token chunks per core
EXPERT_NP = ml_dtypes.bfloat16
EXPERT_DT = BF16

_PROG_CACHE = {}
LAST_RESULTS = None


def _build_program():
    nc = bacc.Bacc("TRN2", target_bir_lowering=False, debug=False,
                   enable_asserts=False, num_devices=NCORE)

    def din(name, shape, dt=F32):
        return nc.dram_tensor(name, list(shape), dt, kind="ExternalInput").ap()

    # per-core inputs
    x_fm_bf_d = din("x_fm_bf", [P, H // P, T], EXPERT_DT)
    x_fm_d = din("x_fm", [P, H // P, T])
    x_tok_d = din("x_tok", [P, TCH, H])
    te_rhs_d = din("te_rhs", [P, A // P, T])
    te_attr_d = din("te_attr", [TD, NT, T])
    # replicated router weights
    R1_d = din("R1", [P, 4 * H // P, 10, P])       # r_in_w^T packed (ki, oc, ko, oi)
    rib_d = din("rib", [P, 4 * H // P])
    R2_d = din("R2", [P, H // P, 32, P])
    rmb_d = din("rmb", [P, H // P])
    RW_d = din("RW", [P, H // P, E])
    rbb_d = din("rbb", [P, E])
    RA_d = din("RA", [TD, E])
    # replicated expert weights (stacked on E)
    W1_d = din("W1", [E, P, H // P, I], EXPERT_DT)
    C1_d = din("C1", [P, E, I // P])
    G2_d = din("G2", [E, P, I // P, H], EXPERT_DT)
    C2_d = din("C2", [P, E, H // P])
    F1_d = din("F1", [E, P, H // P, I], EXPERT_DT)
    B1_d = din("B1", [P, E, I // P])
    F2_d = din("F2", [E, P, I // P, H], EXPERT_DT)
    FB2_d = din("FB2", [E, P, H])
    LG_d = din("LG", [E, P, H])
    LB_d = din("LB", [E, P, H])
    AP_d = din("APc", [P, E, H // P])

    out_d = nc.dram_tensor("out", [P, TCH, H], F32, kind="ExternalOutput").ap()
    ent_d = nc.dram_tensor("ent", [P, TCH], F32, kind="ExternalOutput").ap()

    with tile.TileContext(nc) as tc:
        with ExitStack() as top:
            const = top.enter_context(tc.tile_pool(name="const", bufs=1))
            ps_mm = top.enter_context(tc.tile_pool(name="ps_mm", bufs=2, space="PSUM"))

            x_fm_bf = const.tile([P, H // P, T], EXPERT_DT)
            nc.sync.dma_start(x_fm_bf[:], x_fm_bf_d)
            x_tok = const.tile([P, TCH, H], F32)
            nc.sync.dma_start(x_tok[:], x_tok_d)
            c1_sb = const.tile([P, E, I // P], F32)
            nc.sync.dma_start(c1_sb[:], C1_d)
            c2_sb = const.tile([P, E, H // P], F32)
            nc.sync.dma_start(c2_sb[:], C2_d)
            b1_sb = const.tile([P, E, I // P], F32)
            nc.sync.dma_start(b1_sb[:], B1_d)
            ap_sb = const.tile([P, E, H // P], F32)
            nc.sync.dma_start(ap_sb[:], AP_d)
            probs = const.tile([P, TCH, E], F32)
            ent = const.tile([P, TCH], F32)
            eps8 = const.tile([P, 1], F32)
            nc.vector.memset(eps8[:], 1e-8)

            # ---------------- router (fp32) ----------------
            with ExitStack() as rs:
                rpool = rs.enter_context(tc.tile_pool(name="rconst", bufs=1))
                r1p = rs.enter_context(tc.tile_pool(name="r1", bufs=3))
                r2p = rs.enter_context(tc.tile_pool(name="r2", bufs=2))
                h1p = rs.enter_context(tc.tile_pool(name="h1", bufs=1))
                ps_sm = rs.enter_context(tc.tile_pool(name="ps_sm", bufs=2, space="PSUM"))
                smt = rs.enter_context(tc.tile_pool(name="smt", bufs=4))

                x_fm = rpool.tile([P, H // P, T], F32)
                nc.sync.dma_start(x_fm[:], x_fm_d)
                te_rhs = rpool.tile([P, A // P, T], F32)
                nc.sync.dma_start(te_rhs[:], te_rhs_d)
                te_attr = rpool.tile([TD, NT, T], F32)
                nc.sync.dma_start(te_attr[:], te_attr_d)
                ra_sb = rpool.tile([TD, E], F32)
                nc.sync.dma_start(ra_sb[:], RA_d)
                rib_sb = rpool.tile([P, 4 * H // P], F32)
                nc.sync.dma_start(rib_sb[:], rib_d)
                rmb_sb = rpool.tile([P, H // P], F32)
                nc.sync.dma_start(rmb_sb[:], rmb_d)
                rw_sb = rpool.tile([P, H // P, E], F32)
                nc.sync.dma_start(rw_sb[:], RW_d)
                rbb_sb = rpool.tile([P, E], F32)
                nc.sync.dma_start(rbb_sb[:], rbb_d)

                h1 = h1p.tile([P, 4 * H // P, T], F32)
                h2 = rpool.tile([P, H // P, T], F32)

                rhs_list = [x_fm[:, i] for i in range(H // P)] + \
                           [te_rhs[:, i] for i in range(A // P)]
                # iv1 = relu([x, te] @ r_in_w.T + r_in_b)   [4H, T]
                for oc in range(4 * H // P):
                    w = r1p.tile([P, 10, P], F32, tag="r1w")
                    nc.sync.dma_start(w[:], R1_d[:, oc])
                    ps = ps_mm.tile([P, T], F32, tag="mm")
                    for kt in range(10):
                        nc.tensor.matmul(ps[:], w[:, kt], rhs_list[kt],
                                         start=(kt == 0), stop=(kt == 9))
                    nc.scalar.activation(h1[:, oc], ps[:], AF.Relu,
                                         bias=rib_sb[:, oc:oc + 1])
                # iv2 = relu(iv1 @ r_mid_w.T + r_mid_b)   [H, T]
                for oc in range(H // P):
                    w = r2p.tile([P, 32, P], F32, tag="r2w")
                    nc.sync.dma_start(w[:], R2_d[:, oc])
                    ps = ps_mm.tile([P, T], F32, tag="mm")
                    for kt in range(32):
                        nc.tensor.matmul(ps[:], w[:, kt], h1[:, kt],
                                         start=(kt == 0), stop=(kt == 31))
                    nc.scalar.activation(h2[:, oc], ps[:], AF.Relu,
                                         bias=rmb_sb[:, oc:oc + 1])
                # logits / softmax / attr softmax / top-2 per token chunk
                for tcx in range(TCH):
                    ps = ps_sm.tile([P, E], F32, tag="sm")
                    for kt in range(H // P):
                        nc.tensor.matmul(ps[:], h2[:, kt, ts(tcx, P)], rw_sb[:, kt],
                                         start=(kt == 0), stop=(kt == H // P - 1))
                    lg = smt.tile([P, E], F32, tag="s8")
                    nc.vector.tensor_add(lg[:], ps[:], rbb_sb[:])
                    nmax = smt.tile([P, 1], F32, tag="s1")
                    nc.vector.reduce_max(nmax[:], lg[:], axis=AXX, negate=True)
                    sume = smt.tile([P, 1], F32, tag="s1")
                    ep = smt.tile([P, E], F32, tag="s8")
                    nc.scalar.activation(ep[:], lg[:], AF.Exp, bias=nmax[:],
                                         accum_out=sume[:])
                    rec = smt.tile([P, 1], F32, tag="s1")
                    nc.vector.reciprocal(rec[:], sume[:])
                    nc.vector.tensor_scalar_mul(ep[:], ep[:], rec[:])
                    asum = smt.tile([P, E], F32, tag="s8")
                    for n in range(NT):
                        psa = ps_sm.tile([P, E], F32, tag="sm")
                        nc.tensor.matmul(psa[:], te_attr[:, n, ts(tcx, P)], ra_sb[:],
                                         start=True, stop=True)
                        nma = smt.tile([P, 1], F32, tag="s1")
                        nc.vector.reduce_max(nma[:], psa[:], axis=AXX, negate=True)
                        suma = smt.tile([P, 1], F32, tag="s1")
                        an = smt.tile([P, E], F32, tag="s8")
                        nc.scalar.activation(an[:], psa[:], AF.Exp, bias=nma[:],
                                             accum_out=suma[:])
                        reca = smt.tile([P, 1], F32, tag="s1")
                        nc.vector.reciprocal(reca[:], suma[:])
                        if n == 0:
                            nc.vector.tensor_scalar_mul(asum[:], an[:], reca[:])
                        else:
                            tmp8 = smt.tile([P, E], F32, tag="s8")
                            nc.vector.tensor_scalar_mul(tmp8[:], an[:], reca[:])
                            nc.vector.tensor_add(asum[:], asum[:], tmp8[:])
                    pf = smt.tile([P, E], F32, tag="s8")
                    nc.vector.tensor_mul(pf[:], ep[:], asum[:])
                    nc.vector.tensor_scalar_mul(pf[:], pf[:], 1.0 / NT)
                    # top-2 mask (values >= 2nd max)
                    m1 = smt.tile([P, 1], F32, tag="s1")
                    nc.vector.reduce_max(m1[:], pf[:], axis=AXX)
                    eq = smt.tile([P, E], F32, tag="s8")
                    nc.vector.tensor_single_scalar(eq[:], pf[:], m1[:], ALU.is_ge)
                    nc.vector.tensor_scalar_mul(eq[:], eq[:], -1e30)
                    p2t = smt.tile([P, E], F32, tag="s8")
                    nc.vector.tensor_add(p2t[:], pf[:], eq[:])
                    m2 = smt.tile([P, 1], F32, tag="s1")
                    nc.vector.reduce_max(m2[:], p2t[:], axis=AXX)
                    msk = smt.tile([P, E], F32, tag="s8")
                    nc.vector.tensor_single_scalar(msk[:], pf[:], m2[:], ALU.is_ge)
                    nc.vector.tensor_mul(probs[:, tcx], pf[:], msk[:])
                    # entropy partial: sum_e p*log(p+1e-8)
                    l8 = smt.tile([P, E], F32, tag="s8")
                    nc.scalar.activation(l8[:], probs[:, tcx], AF.Ln, bias=eps8[:])
                    scr = smt.tile([P, E], F32, tag="s8")
                    nc.vector.tensor_mul(scr[:], probs[:, tcx], l8[:])
                    nc.vector.reduce_sum(ent[:, tcx:tcx + 1], scr[:], axis=AXX)
                nc.sync.dma_start(ent_d, ent[:])

            # ---------------- experts (bf16) ----------------
            with ExitStack() as es:
                wp = es.enter_context(tc.tile_pool(name="w", bufs=2))
                zp = es.enter_context(tc.tile_pool(name="z", bufs=2))
                gp = es.enter_context(tc.tile_pool(name="g", bufs=3))
                hp = es.enter_context(tc.tile_pool(name="h", bufs=2))
                lnp = es.enter_context(tc.tile_pool(name="ln", bufs=2))
                ept = es.enter_context(tc.tile_pool(name="ept", bufs=3))
                sp = es.enter_context(tc.tile_pool(name="sp", bufs=6))
                ps_out = es.enter_context(tc.tile_pool(name="ps_out", bufs=4, space="PSUM"))

                for e in range(E):
                    # z1 = relu(x @ W1x.T + c1)    [I, T]
                    w1 = wp.tile([P, H // P, I], EXPERT_DT, tag="w")
                    nc.sync.dma_start(w1[:], W1_d[e])
                    z1 = zp.tile([P, I // P, T], EXPERT_DT, tag="z")
                    for oc in range(I // P):
                        ps = ps_mm.tile([P, T], F32, tag="mm")
                        for kt in range(H // P):
                            nc.tensor.matmul(ps[:], w1[:, kt, ts(oc, P)], x_fm_bf[:, kt],
                                             start=(kt == 0), stop=(kt == H // P - 1))
                        nc.scalar.activation(z1[:, oc], ps[:], AF.Relu,
                                             bias=c1_sb[:, e, oc:oc + 1])
                    # gate = sigmoid(z1 @ gw2.T + gb2); h = gate*(x-ap)+ap  [H, T]
                    g2 = wp.tile([P, I // P, H], EXPERT_DT, tag="w")
                    nc.sync.dma_start(g2[:], G2_d[e])
                    hb = hp.tile([P, H // P, T], EXPERT_DT, tag="h")
                    for oc in range(H // P):
                        ps = ps_mm.tile([P, T], F32, tag="mm")
                        for kt in range(I // P):
                            nc.tensor.matmul(ps[:], g2[:, kt, ts(oc, P)], z1[:, kt],
                                             start=(kt == 0), stop=(kt == I // P - 1))
                        gt = gp.tile([P, T], F32, tag="gt")
                        nc.scalar.activation(gt[:], ps[:], AF.Sigmoid,
                                             bias=c2_sb[:, e, oc:oc + 1])
                        xm = gp.tile([P, T], F32, tag="xm")
                        nc.vector.tensor_single_scalar(xm[:], x_fm_bf[:, oc],
                                                       ap_sb[:, e, oc:oc + 1], ALU.subtract)
                        nc.vector.tensor_mul(xm[:], xm[:], gt[:])
                        nc.vector.tensor_single_scalar(hb[:, oc], xm[:],
                                                       ap_sb[:, e, oc:oc + 1], ALU.add)
                    # z2 = relu(h @ fw1.T + fb1)   [I, T]
                    f1 = wp.tile([P, H // P, I], EXPERT_DT, tag="w")
                    nc.sync.dma_start(f1[:], F1_d[e])
                    z2 = zp.tile([P, I // P, T], EXPERT_DT, tag="z")
                    for oc in range(I // P):
                        ps = ps_mm.tile([P, T], F32, tag="mm")
                        for kt in range(H // P):
                            nc.tensor.matmul(ps[:], f1[:, kt, ts(oc, P)], hb[:, kt],
                                             start=(kt == 0), stop=(kt == H // P - 1))
                        nc.scalar.activation(z2[:, oc], ps[:], AF.Relu,
                                             bias=b1_sb[:, e, oc:oc + 1])
                    # o = z2 @ fw2.T + fb2 (token-major out), r = o + x,
                    # y = LN(r)*g+b, out += prob_e * y
                    f2 = wp.tile([P, I // P, H], EXPERT_DT, tag="w")
                    nc.sync.dma_start(f2[:], F2_d[e])
                    fb2_sb = lnp.tile([P, H], F32, tag="fb2")
                    nc.sync.dma_start(fb2_sb[:], FB2_d[e])
                    lg_sb = lnp.tile([P, H], F32, tag="lg")
                    nc.sync.dma_start(lg_sb[:], LG_d[e])
                    lb_sb = lnp.tile([P, H], F32, tag="lb")
                    nc.sync.dma_start(lb_sb[:], LB_d[e])
                    for tcx in range(TCH):
                        t1 = ept.tile([P, H], F32, tag="t")
                        for hh in range(H // 512):
                            pso = ps_out.tile([P, 512], F32, tag="out")
                            for kt in range(I // P):
                                nc.tensor.matmul(pso[:],
                                                 z2[:, kt, ts(tcx, P)],
                                                 f2[:, kt, ts(hh, 512)],
                                                 start=(kt == 0), stop=(kt == I // P - 1))
                            nc.vector.tensor_add(t1[:, ts(hh, 512)], pso[:],
                                                 fb2_sb[:, ts(hh, 512)])
                        r = ept.tile([P, H], F32, tag="t")
                        s1 = sp.tile([P, 1], F32, tag="sp")
                        nc.vector.tensor_add(r[:], t1[:], x_tok[:, tcx])
                        nc.vector.reduce_sum(s1[:], r[:], axis=AXX)
                        s2 = sp.tile([P, 1], F32, tag="sp")
                        nc.scalar.activation(t1[:], r[:], AF.Square, accum_out=s2[:])
                        mu = sp.tile([P, 1], F32, tag="sp")
                        nc.vector.tensor_scalar_mul(mu[:], s1[:], 1.0 / H)
                        nm = sp.tile([P, 1], F32, tag="sp")
                        nc.vector.tensor_mul(nm[:], mu[:], mu[:])
                        nc.vector.tensor_scalar(nm[:], nm[:], -1.0, 1e-5, ALU.mult, ALU.add)
                        std = sp.tile([P, 1], F32, tag="sp")
                        nc.scalar.activation(std[:], s2[:], AF.Sqrt, bias=nm[:],
                                             scale=1.0 / H)
                        rstd = sp.tile([P, 1], F32, tag="sp")
                        nc.vector.reciprocal(rstd[:], std[:])
                        yn = ept.tile([P, H], F32, tag="t")
                        nc.vector.tensor_scalar(yn[:], r[:], mu[:], rstd[:],
                                                ALU.subtract, ALU.mult)
                        nc.vector.tensor_mul(yn[:], yn[:], lg_sb[:])
                        nc.vector.tensor_add(yn[:], yn[:], lb_sb[:])
                        wy = ept.tile([P, H], F32, tag="t")
                        nc.vector.tensor_single_scalar(wy[:], yn[:],
                                                       probs[:, tcx, e:e + 1], ALU.mult)
                        if e == 0:
                            nc.sync.dma_start(out_d[:, tcx], wy[:])
                        else:
                            nc.gpsimd.dma_start(out_d[:, tcx], wy[:], accum_op=ALU.add)

    nc.compile()
    return nc


def _pack_weights(i):
    f32 = np.float32
    gw1 = np.asarray(i["gate_w1"], f32)
    gb1 = np.asarray(i["gate_b1"], f32)
    gw2 = np.asarray(i["gate_w2"], f32)
    gb2 = np.asarray(i["gate_b2"], f32)
    fw1 = np.asarray(i["fc_w1"], f32)
    fb1 = np.asarray(i["fc_b1"], f32)
    fw2 = np.asarray(i["fc_w2"], f32)
    fb2 = np.asarray(i["fc_b2"], f32)
    ln_g = np.asarray(i["ln_g"], f32)
    ln_b = np.asarray(i["ln_b"], f32)
    ae = np.asarray(i["attr_emb"], f32)
    apw = np.asarray(i["attr_proj_w"], f32)
    apb = np.asarray(i["attr_proj_b"], f32)
    r_attr = np.asarray(i["r_attr"], f32)
    r_in_w = np.asarray(i["r_in_w"], f32)
    r_in_b = np.asarray(i["r_in_b"], f32)
    r_mid_w = np.asarray(i["r_mid_w"], f32)
    r_mid_b = np.asarray(i["r_mid_b"], f32)
    r_w = np.asarray(i["r_w"], f32)
    r_b = np.asarray(i["r_b"], f32)

    C = np.ascontiguousarray
    # input-independent precomputation
    ap_vec = np.einsum("ea,eha->eh", ae[:, 0], apw) + apb           # [E,H]
    c1 = np.einsum("eh,eih->ei", ap_vec, gw1[:, :, H:]) + gb1       # [E,I]
    W1x = gw1[:, :, :H]                                             # [E,I,H]

    def pack_lhsT(wT):  # [K,O] -> [P, K//P, O]
        K, O = wT.shape
        return C(wT.reshape(K // P, P, O).transpose(1, 0, 2))

    def chunks(v, nch):  # [E, nch*P] -> [P, E, nch]
        return C(np.stack([v[e].reshape(nch, P).T for e in range(E)], axis=1))

    d = {}
    r_in_wT = C(r_in_w.T)     # [1280, 4096]
    d["R1"] = C(r_in_wT.reshape(10, P, 32, P).transpose(1, 2, 0, 3))
    d["rib"] = C(r_in_b.reshape(32, P).T)
    r_mid_wT = C(r_mid_w.T)   # [4096, 1024]
    d["R2"] = C(r_mid_wT.reshape(32, P, 8, P).transpose(1, 2, 0, 3))
    d["rmb"] = C(r_mid_b.reshape(8, P).T)
    d["RW"] = C(r_w.T.reshape(8, P, E).transpose(1, 0, 2))
    d["rbb"] = C(np.tile(r_b, (P, 1)))
    d["RA"] = C(r_attr)
    d["W1"] = np.stack([pack_lhsT(C(W1x[e].T)) for e in range(E)]).astype(EXPERT_NP)
    d["C1"] = chunks(c1, I // P)
    d["G2"] = np.stack([pack_lhsT(C(gw2[e].T)) for e in range(E)]).astype(EXPERT_NP)
    d["C2"] = chunks(gb2, H // P)
    d["F1"] = np.stack([pack_lhsT(C(fw1[e].T)) for e in range(E)]).astype(EXPERT_NP)
    d["B1"] = chunks(fb1, I // P)
    d["F2"] = np.stack([pack_lhsT(C(fw2[e].T)) for e in range(E)]).astype(EXPERT_NP)
    d["FB2"] = C(np.tile(fb2[:, None, :], (1, P, 1)))
    d["LG"] = C(np.tile(ln_g[:, None, :], (1, P, 1)))
    d["LB"] = C(np.tile(ln_b[:, None, :], (1, P, 1)))
    d["APc"] = chunks(ap_vec, H // P)
    return d


def _pack_core_inputs(x, te, c):
    C = np.ascontiguousarray
    sl = slice(c * T, (c + 1) * T)
    xs = x.reshape(NTOK, H)[sl]                    # [T,H]
    te_s = te.reshape(NTOK, NT, TD)[sl]            # [T,NT,TD]
    xT = C(xs.T)                                   # [H,T]
    x_fm = C(xT.reshape(H // P, P, T).transpose(1, 0, 2))
    d = {
        "x_fm": x_fm,
        "x_fm_bf": x_fm.astype(EXPERT_NP),
        "x_tok": C(xs.reshape(TCH, P, H).transpose(1, 0, 2)),
        "te_rhs": C(te_s.reshape(T, A).T.reshape(A // P, P, T).transpose(1, 0, 2)),
        "te_attr": C(te_s.transpose(2, 1, 0)),
    }
    return d


def run_full(inputs, trace=False):
    global LAST_RESULTS
    x = np.asarray(inputs["x"], np.float32)
    te = np.asarray(inputs["task_embeddings"], np.float32)

    if "prog" not in _PROG_CACHE:
        _PROG_CACHE["prog"] = _build_program()
    nc = _PROG_CACHE["prog"]

    wmap = _pack_weights(inputs)
    in_maps = []
    for c in range(NCORE):
        m = dict(wmap)
        m.update(_pack_core_inputs(x, te, c))
        in_maps.append(m)

    res = run_bass_kernel_spmd(nc, in_maps, core_ids=list(range(NCORE)), trace=trace)
    LAST_RESULTS = res

    out = np.empty((NTOK, H), np.float32)
    ent_total = np.float64(0.0)
    for c in range(NCORE):
        oc = res.results[c]["out"]                 # [P, TCH, H]
        out[c * T:(c + 1) * T] = oc.transpose(1, 0, 2).reshape(T, H)
        ent_total += np.float64(res.results[c]["ent"].sum(dtype=np.float64))
    loss = np.float32(-(ent_total / NTOK))
    return out.reshape(B, S, H), loss


def kernel(**inputs):
    return run_full(inputs, trace=False)


# revision 13
# speedup vs baseline: 1.2737x; 1.2737x over previous
# MoE layer (8 experts, top-2 routing) on 8 trn2 NeuronCores.
# Sharding: token-parallel. Each core processes 512 of the 4096 tokens:
# full router + all 8 experts (dense eval, matching the reference) on its
# token slice. No collectives; host concatenates the per-core outputs.
#
# Layouts: activations are feature-major [feat_part, tok_free] through the
# MLPs (contraction dim always on partitions -> no transposes); the last
# expert matmul (fc2) swaps lhsT/rhs so its output lands token-major
# [tok_part, feat_free], where layernorm + the prob-weighted combine only
# need per-partition scalars. The input-independent attr projection and the
# ap-half of gate_w1 are folded into host-precomputed biases.
import numpy as np
import ml_dtypes
from contextlib import ExitStack

import concourse.bass as bass
import concourse.bacc as bacc
import concourse.mybir as mybir
import concourse.tile as tile
from concourse.bass import ts
from concourse.bass_utils import run_bass_kernel_spmd

F32 = mybir.dt.float32
F32R = mybir.dt.float32r
BF16 = mybir.dt.bfloat16
ROUTER_F32R = True   # run the two big router matmul layers as float32r
RDT = F32R if ROUTER_F32R else F32
AF = mybir.ActivationFunctionType
ALU = mybir.AluOpType
AXX = mybir.AxisListType.X

H, E, TD, NT, TOPK, I = 1024, 8, 64, 4, 2, 2048
A = TD * NT
B, S = 4, 1024
NTOK = B * S
NCORE = 8
T = NTOK // NCORE          # tokens per core
P = 128
TCH = T // P               # token chunks per core
EXPERT_NP = ml_dtypes.bfloat16
EXPERT_DT = BF16

_PROG_CACHE = {}
LAST_RESULTS = None


def _build_program():
    nc = bacc.Bacc("TRN2", target_bir_lowering=False, debug=False,
                   enable_asserts=False, num_devices=NCORE)

    def din(name, shape, dt=F32):
        return nc.dram_tensor(name, list(shape), dt, kind="ExternalInput").ap()

    # per-core inputs
    x_fm_bf_d = din("x_fm_bf", [P, H // P, T], EXPERT_DT)
    x_fm_d = din("x_fm", [P, H // P, T], RDT)
    x_tok_d = din("x_tok", [P, TCH, H])
    te_rhs_d = din("te_rhs", [P, A // P, T], RDT)
    te_attr_d = din("te_attr", [TD, NT, T])
    # replicated router weights
    R1_d = din("R1", [P, 4 * H // P, 10, P], RDT)       # r_in_w^T packed (ki, oc, ko, oi)
    rib_d = din("rib", [P, 4 * H // P])
    R2_d = din("R2", [P, H // P, 32, P], RDT)
    rmb_d = din("rmb", [P, H // P])
    RW_d = din("RW", [P, H // P, E])
    rbb_d = din("rbb", [P, E])
    RA_d = din("RA", [TD, E])
    # replicated expert weights (stacked on E)
    W1_d = din("W1", [E, P, H // P, I], EXPERT_DT)
    C1_d = din("C1", [P, E, I // P])
    G2_d = din("G2", [E, P, I // P, H], EXPERT_DT)
    C2_d = din("C2", [P, E, H // P])
    F1_d = din("F1", [E, P, H // P, I], EXPERT_DT)
    B1_d = din("B1", [P, E, I // P])
    F2_d = din("F2", [E, P, I // P, H], EXPERT_DT)
    FB2_d = din("FB2", [E, P, H])
    LG_d = din("LG", [E, P, H])
    LB_d = din("LB", [E, P, H])
    AP_d = din("APc", [P, E, H // P])

    out_d = nc.dram_tensor("out", [P, TCH, H], F32, kind="ExternalOutput").ap()
    ent_d = nc.dram_tensor("ent", [P, TCH], F32, kind="ExternalOutput").ap()

    with tile.TileContext(nc) as tc:
        with ExitStack() as top:
            const = top.enter_context(tc.tile_pool(name="const", bufs=1))
            ps_mm = top.enter_context(tc.tile_pool(name="ps_mm", bufs=2, space="PSUM"))

            x_fm_bf = const.tile([P, H // P, T], EXPERT_DT)
            nc.sync.dma_start(x_fm_bf[:], x_fm_bf_d)
            x_tok = const.tile([P, TCH, H], F32)
            nc.sync.dma_start(x_tok[:], x_tok_d)
            c1_sb = const.tile([P, E, I // P], F32)
            nc.sync.dma_start(c1_sb[:], C1_d)
            c2_sb = const.tile([P, E, H // P], F32)
            nc.sync.dma_start(c2_sb[:], C2_d)
            b1_sb = const.tile([P, E, I // P], F32)
            nc.sync.dma_start(b1_sb[:], B1_d)
            ap_sb = const.tile([P, E, H // P], F32)
            nc.sync.dma_start(ap_sb[:], AP_d)
            probs = const.tile([P, TCH, E], F32)
            ent = const.tile([P, TCH], F32)
            eps8 = const.tile([P, 1], F32)
            nc.vector.memset(eps8[:], 1e-8)

            # ---------------- router (fp32) ----------------
            with ExitStack() as rs:
                rpool = rs.enter_context(tc.tile_pool(name="rconst", bufs=1))
                r1p = rs.enter_context(tc.tile_pool(name="r1", bufs=3))
                r2p = rs.enter_context(tc.tile_pool(name="r2", bufs=2))
                h1p = rs.enter_context(tc.tile_pool(name="h1", bufs=1))
                ps_sm = rs.enter_context(tc.tile_pool(name="ps_sm", bufs=2, space="PSUM"))
                smt = rs.enter_context(tc.tile_pool(name="smt", bufs=4))

                x_fm = rpool.tile([P, H // P, T], RDT)
                nc.sync.dma_start(x_fm[:], x_fm_d)
                te_rhs = rpool.tile([P, A // P, T], RDT)
                nc.sync.dma_start(te_rhs[:], te_rhs_d)
                te_attr = rpool.tile([TD, NT, T], F32)
                nc.sync.dma_start(te_attr[:], te_attr_d)
                ra_sb = rpool.tile([TD, E], F32)
                nc.sync.dma_start(ra_sb[:], RA_d)
                rib_sb = rpool.tile([P, 4 * H // P], F32)
                nc.sync.dma_start(rib_sb[:], rib_d)
                rmb_sb = rpool.tile([P, H // P], F32)
                nc.sync.dma_start(rmb_sb[:], rmb_d)
                rw_sb = rpool.tile([P, H // P, E], F32)
                nc.sync.dma_start(rw_sb[:], RW_d)
                rbb_sb = rpool.tile([P, E], F32)
                nc.sync.dma_start(rbb_sb[:], rbb_d)

                h1 = h1p.tile([P, 4 * H // P, T], RDT)
                h2 = rpool.tile([P, H // P, T], F32)

                rhs_list = [x_fm[:, i] for i in range(H // P)] + \
                           [te_rhs[:, i] for i in range(A // P)]
                # iv1 = relu([x, te] @ r_in_w.T + r_in_b)   [4H, T]
                for oc in range(4 * H // P):
                    w = r1p.tile([P, 10, P], RDT, tag="r1w")
                    nc.sync.dma_start(w[:], R1_d[:, oc])
                    ps = ps_mm.tile([P, T], F32, tag="mm")
                    for kt in range(10):
                        nc.tensor.matmul(ps[:], w[:, kt], rhs_list[kt],
                                         start=(kt == 0), stop=(kt == 9))
                    nc.scalar.activation(h1[:, oc], ps[:], AF.Relu,
                                         bias=rib_sb[:, oc:oc + 1])
                # iv2 = relu(iv1 @ r_mid_w.T + r_mid_b)   [H, T]
                for oc in range(H // P):
                    w = r2p.tile([P, 32, P], RDT, tag="r2w")
                    nc.sync.dma_start(w[:], R2_d[:, oc])
                    ps = ps_mm.tile([P, T], F32, tag="mm")
                    for kt in range(32):
                        nc.tensor.matmul(ps[:], w[:, kt], h1[:, kt],
                                         start=(kt == 0), stop=(kt == 31))
                    nc.scalar.activation(h2[:, oc], ps[:], AF.Relu,
                                         bias=rmb_sb[:, oc:oc + 1])
                # logits / softmax / attr softmax / top-2 per token chunk
                for tcx in range(TCH):
                    ps = ps_sm.tile([P, E], F32, tag="sm")
                    for kt in range(H // P):
                        nc.tensor.matmul(ps[:], h2[:, kt, ts(tcx, P)], rw_sb[:, kt],
                                         start=(kt == 0), stop=(kt == H // P - 1))
                    lg = smt.tile([P, E], F32, tag="s8")
                    nc.vector.tensor_add(lg[:], ps[:], rbb_sb[:])
                    nmax = smt.tile([P, 1], F32, tag="s1")
                    nc.vector.reduce_max(nmax[:], lg[:], axis=AXX, negate=True)
                    sume = smt.tile([P, 1], F32, tag="s1")
                    ep = smt.tile([P, E], F32, tag="s8")
                    nc.scalar.activation(ep[:], lg[:], AF.Exp, bias=nmax[:],
                                         accum_out=sume[:])
                    rec = smt.tile([P, 1], F32, tag="s1")
                    nc.vector.reciprocal(rec[:], sume[:])
                    nc.vector.tensor_scalar_mul(ep[:], ep[:], rec[:])
                    asum = smt.tile([P, E], F32, tag="s8")
                    for n in range(NT):
                        psa = ps_sm.tile([P, E], F32, tag="sm")
                        nc.tensor.matmul(psa[:], te_attr[:, n, ts(tcx, P)], ra_sb[:],
                                         start=True, stop=True)
                        nma = smt.tile([P, 1], F32, tag="s1")
                        nc.vector.reduce_max(nma[:], psa[:], axis=AXX, negate=True)
                        suma = smt.tile([P, 1], F32, tag="s1")
                        an = smt.tile([P, E], F32, tag="s8")
                        nc.scalar.activation(an[:], psa[:], AF.Exp, bias=nma[:],
                                             accum_out=suma[:])
                        reca = smt.tile([P, 1], F32, tag="s1")
                        nc.vector.reciprocal(reca[:], suma[:])
                        if n == 0:
                            nc.vector.tensor_scalar_mul(asum[:], an[:], reca[:])
                        else:
                            tmp8 = smt.tile([P, E], F32, tag="s8")
                            nc.vector.tensor_scalar_mul(tmp8[:], an[:], reca[:])
                            nc.vector.tensor_add(asum[:], asum[:], tmp8[:])
                    pf = smt.tile([P, E], F32, tag="s8")
                    nc.vector.tensor_mul(pf[:], ep[:], asum[:])
                    nc.vector.tensor_scalar_mul(pf[:], pf[:], 1.0 / NT)
                    # top-2 mask (values >= 2nd max)
                    m1 = smt.tile([P, 1], F32, tag="s1")
                    nc.vector.reduce_max(m1[:], pf[:], axis=AXX)
                    eq = smt.tile([P, E], F32, tag="s8")
                    nc.vector.tensor_single_scalar(eq[:], pf[:], m1[:], ALU.is_ge)
                    nc.vector.tensor_scalar_mul(eq[:], eq[:], -1e30)
                    p2t = smt.tile([P, E], F32, tag="s8")
                    nc.vector.tensor_add(p2t[:], pf[:], eq[:])
                    m2 = smt.tile([P, 1], F32, tag="s1")
                    nc.vector.reduce_max(m2[:], p2t[:], axis=AXX)
                    msk = smt.tile([P, E], F32, tag="s8")
                    nc.vector.tensor_single_scalar(msk[:], pf[:], m2[:], ALU.is_ge)
                    nc.vector.tensor_mul(probs[:, tcx], pf[:], msk[:])
                    # entropy partial: sum_e p*log(p+1e-8)
                    l8 = smt.tile([P, E], F32, tag="s8")
                    nc.scalar.activation(l8[:], probs[:, tcx], AF.Ln, bias=eps8[:])
                    scr = smt.tile([P, E], F32, tag="s8")
                    nc.vector.tensor_mul(scr[:], probs[:, tcx], l8[:])
                    nc.vector.reduce_sum(ent[:, tcx:tcx + 1], scr[:], axis=AXX)
                nc.sync.dma_start(ent_d, ent[:])

            # ---------------- experts (bf16) ----------------
            with ExitStack() as es:
                wp = es.enter_context(tc.tile_pool(name="w", bufs=2))
                zp = es.enter_context(tc.tile_pool(name="z", bufs=2))
                gp = es.enter_context(tc.tile_pool(name="g", bufs=3))
                hp = es.enter_context(tc.tile_pool(name="h", bufs=2))
                lnp = es.enter_context(tc.tile_pool(name="ln", bufs=2))
                ept = es.enter_context(tc.tile_pool(name="ept", bufs=3))
                sp = es.enter_context(tc.tile_pool(name="sp", bufs=6))
                ps_out = es.enter_context(tc.tile_pool(name="ps_out", bufs=4, space="PSUM"))

                for e in range(E):
                    # z1 = relu(x @ W1x.T + c1)    [I, T]
                    w1 = wp.tile([P, H // P, I], EXPERT_DT, tag="w")
                    nc.sync.dma_start(w1[:], W1_d[e])
                    z1 = zp.tile([P, I // P, T], EXPERT_DT, tag="z")
                    for oc in range(I // P):
                        ps = ps_mm.tile([P, T], F32, tag="mm")
                        for kt in range(H // P):
                            nc.tensor.matmul(ps[:], w1[:, kt, ts(oc, P)], x_fm_bf[:, kt],
                                             start=(kt == 0), stop=(kt == H // P - 1))
                        nc.scalar.activation(z1[:, oc], ps[:], AF.Relu,
                                             bias=c1_sb[:, e, oc:oc + 1])
                    # gate = sigmoid(z1 @ gw2.T + gb2); h = gate*(x-ap)+ap  [H, T]
                    g2 = wp.tile([P, I // P, H], EXPERT_DT, tag="w")
                    nc.sync.dma_start(g2[:], G2_d[e])
                    hb = hp.tile([P, H // P, T], EXPERT_DT, tag="h")
                    for oc in range(H // P):
                        ps = ps_mm.tile([P, T], F32, tag="mm")
                        for kt in range(I // P):
                            nc.tensor.matmul(ps[:], g2[:, kt, ts(oc, P)], z1[:, kt],
                                             start=(kt == 0), stop=(kt == I // P - 1))
                        gt = gp.tile([P, T], F32, tag="gt")
                        nc.scalar.activation(gt[:], ps[:], AF.Sigmoid,
                                             bias=c2_sb[:, e, oc:oc + 1])
                        xm = gp.tile([P, T], F32, tag="xm")
                        nc.vector.tensor_single_scalar(xm[:], x_fm_bf[:, oc],
                                                       ap_sb[:, e, oc:oc + 1], ALU.subtract)
                        nc.vector.tensor_mul(xm[:], xm[:], gt[:])
                        nc.vector.tensor_single_scalar(hb[:, oc], xm[:],
                                                       ap_sb[:, e, oc:oc + 1], ALU.add)
                    # z2 = relu(h @ fw1.T + fb1)   [I, T]
                    f1 = wp.tile([P, H // P, I], EXPERT_DT, tag="w")
                    nc.sync.dma_start(f1[:], F1_d[e])
                    z2 = zp.tile([P, I // P, T], EXPERT_DT, tag="z")
                    for oc in range(I // P):
                        ps = ps_mm.tile([P, T], F32, tag="mm")
                        for kt in range(H // P):
                            nc.tensor.matmul(ps[:], f1[:, kt, ts(oc, P)], hb[:, kt],
                                             start=(kt == 0), stop=(kt == H // P - 1))
                        nc.scalar.activation(z2[:, oc], ps[:], AF.Relu,
                                             bias=b1_sb[:, e, oc:oc + 1])
                    # o = z2 @ fw2.T + fb2 (token-major out), r = o + x,
                    # y = LN(r)*g+b, out += prob_e * y
                    f2 = wp.tile([P, I // P, H], EXPERT_DT, tag="w")
                    nc.sync.dma_start(f2[:], F2_d[e])
                    fb2_sb = lnp.tile([P, H], F32, tag="fb2")
                    nc.sync.dma_start(fb2_sb[:], FB2_d[e])
                    lg_sb = lnp.tile([P, H], F32, tag="lg")
                    nc.sync.dma_start(lg_sb[:], LG_d[e])
                    lb_sb = lnp.tile([P, H], F32, tag="lb")
                    nc.sync.dma_start(lb_sb[:], LB_d[e])
                    for tcx in range(TCH):
                        t1 = ept.tile([P, H], F32, tag="t")
                        for hh in range(H // 512):
                            pso = ps_out.tile([P, 512], F32, tag="out")
                            for kt in range(I // P):
                                nc.tensor.matmul(pso[:],
                                                 z2[:, kt, ts(tcx, P)],
                                                 f2[:, kt, ts(hh, 512)],
                                                 start=(kt == 0), stop=(kt == I // P - 1))
                            nc.vector.tensor_add(t1[:, ts(hh, 512)], pso[:],
                                                 fb2_sb[:, ts(hh, 512)])
                        r = ept.tile([P, H], F32, tag="t")
                        s1 = sp.tile([P, 1], F32, tag="sp")
                        nc.vector.tensor_add(r[:], t1[:], x_tok[:, tcx])
                        nc.vector.reduce_sum(s1[:], r[:], axis=AXX)
                        s2 = sp.tile([P, 1], F32, tag="sp")
                        nc.scalar.activation(t1[:], r[:], AF.Square, accum_out=s2[:])
                        mu = sp.tile([P, 1], F32, tag="sp")
                        nc.vector.tensor_scalar_mul(mu[:], s1[:], 1.0 / H)
                        nm = sp.tile([P, 1], F32, tag="sp")
                        nc.vector.tensor_mul(nm[:], mu[:], mu[:])
                        nc.vector.tensor_scalar(nm[:], nm[:], -1.0, 1e-5, ALU.mult, ALU.add)
                        std = sp.tile([P, 1], F32, tag="sp")
                        nc.scalar.activation(std[:], s2[:], AF.Sqrt, bias=nm[:],
                                             scale=1.0 / H)
                        rstd = sp.tile([P, 1], F32, tag="sp")
                        nc.vector.reciprocal(rstd[:], std[:])
                        yn = ept.tile([P, H], F32, tag="t")
                        nc.vector.tensor_scalar(yn[:], r[:], mu[:], rstd[:],
                                                ALU.subtract, ALU.mult)
                        nc.vector.tensor_mul(yn[:], yn[:], lg_sb[:])
                        nc.vector.tensor_add(yn[:], yn[:], lb_sb[:])
                        wy = ept.tile([P, H], F32, tag="t")
                        nc.vector.tensor_single_scalar(wy[:], yn[:],
                                                       probs[:, tcx, e:e + 1], ALU.mult)
                        if e == 0:
                            nc.sync.dma_start(out_d[:, tcx], wy[:])
                        else:
                            nc.gpsimd.dma_start(out_d[:, tcx], wy[:], accum_op=ALU.add)

    nc.compile()
    return nc


def _pack_weights(i):
    f32 = np.float32
    gw1 = np.asarray(i["gate_w1"], f32)
    gb1 = np.asarray(i["gate_b1"], f32)
    gw2 = np.asarray(i["gate_w2"], f32)
    gb2 = np.asarray(i["gate_b2"], f32)
    fw1 = np.asarray(i["fc_w1"], f32)
    fb1 = np.asarray(i["fc_b1"], f32)
    fw2 = np.asarray(i["fc_w2"], f32)
    fb2 = np.asarray(i["fc_b2"], f32)
    ln_g = np.asarray(i["ln_g"], f32)
    ln_b = np.asarray(i["ln_b"], f32)
    ae = np.asarray(i["attr_emb"], f32)
    apw = np.asarray(i["attr_proj_w"], f32)
    apb = np.asarray(i["attr_proj_b"], f32)
    r_attr = np.asarray(i["r_attr"], f32)
    r_in_w = np.asarray(i["r_in_w"], f32)
    r_in_b = np.asarray(i["r_in_b"], f32)
    r_mid_w = np.asarray(i["r_mid_w"], f32)
    r_mid_b = np.asarray(i["r_mid_b"], f32)
    r_w = np.asarray(i["r_w"], f32)
    r_b = np.asarray(i["r_b"], f32)

    C = np.ascontiguousarray
    # input-independent precomputation
    ap_vec = np.einsum("ea,eha->eh", ae[:, 0], apw) + apb           # [E,H]
    c1 = np.einsum("eh,eih->ei", ap_vec, gw1[:, :, H:]) + gb1       # [E,I]
    W1x = gw1[:, :, :H]                                             # [E,I,H]

    def pack_lhsT(wT):  # [K,O] -> [P, K//P, O]
        K, O = wT.shape
        return C(wT.reshape(K // P, P, O).transpose(1, 0, 2))

    def chunks(v, nch):  # [E, nch*P] -> [P, E, nch]
        return C(np.stack([v[e].reshape(nch, P).T for e in range(E)], axis=1))

    d = {}
    r_in_wT = C(r_in_w.T)     # [1280, 4096]
    d["R1"] = C(r_in_wT.reshape(10, P, 32, P).transpose(1, 2, 0, 3))
    d["rib"] = C(r_in_b.reshape(32, P).T)
    r_mid_wT = C(r_mid_w.T)   # [4096, 1024]
    d["R2"] = C(r_mid_wT.reshape(32, P, 8, P).transpose(1, 2, 0, 3))
    d["rmb"] = C(r_mid_b.reshape(8, P).T)
    d["RW"] = C(r_w.T.reshape(8, P, E).transpose(1, 0, 2))
    d["rbb"] = C(np.tile(r_b, (P, 1)))
    d["RA"] = C(r_attr)
    d["W1"] = np.stack([pack_lhsT(C(W1x[e].T)) for e in range(E)]).astype(EXPERT_NP)
    d["C1"] = chunks(c1, I // P)
    d["G2"] = np.stack([pack_lhsT(C(gw2[e].T)) for e in range(E)]).astype(EXPERT_NP)
    d["C2"] = chunks(gb2, H // P)
    d["F1"] = np.stack([pack_lhsT(C(fw1[e].T)) for e in range(E)]).astype(EXPERT_NP)
    d["B1"] = chunks(fb1, I // P)
    d["F2"] = np.stack([pack_lhsT(C(fw2[e].T)) for e in range(E)]).astype(EXPERT_NP)
    d["FB2"] = C(np.tile(fb2[:, None, :], (1, P, 1)))
    d["LG"] = C(np.tile(ln_g[:, None, :], (1, P, 1)))
    d["LB"] = C(np.tile(ln_b[:, None, :], (1, P, 1)))
    d["APc"] = chunks(ap_vec, H // P)
    return d


def _pack_core_inputs(x, te, c):
    C = np.ascontiguousarray
    sl = slice(c * T, (c + 1) * T)
    xs = x.reshape(NTOK, H)[sl]                    # [T,H]
    te_s = te.reshape(NTOK, NT, TD)[sl]            # [T,NT,TD]
    xT = C(xs.T)                                   # [H,T]
    x_fm = C(xT.reshape(H // P, P, T).transpose(1, 0, 2))
    d = {
        "x_fm": x_fm,
        "x_fm_bf": x_fm.astype(EXPERT_NP),
        "x_tok": C(xs.reshape(TCH, P, H).transpose(1, 0, 2)),
        "te_rhs": C(te_s.reshape(T, A).T.reshape(A // P, P, T).transpose(1, 0, 2)),
        "te_attr": C(te_s.transpose(2, 1, 0)),
    }
    return d


def run_full(inputs, trace=False):
    global LAST_RESULTS
    x = np.asarray(inputs["x"], np.float32)
    te = np.asarray(inputs["task_embeddings"], np.float32)

    if "prog" not in _PROG_CACHE:
        _PROG_CACHE["prog"] = _build_program()
    nc = _PROG_CACHE["prog"]

    wmap = _pack_weights(inputs)
    in_maps = []
    for c in range(NCORE):
        m = dict(wmap)
        m.update(_pack_core_inputs(x, te, c))
        in_maps.append(m)

    res = run_bass_kernel_spmd(nc, in_maps, core_ids=list(range(NCORE)), trace=trace)
    LAST_RESULTS = res

    out = np.empty((NTOK, H), np.float32)
    ent_total = np.float64(0.0)
    for c in range(NCORE):
        oc = res.results[c]["out"]                 # [P, TCH, H]
        out[c * T:(c + 1) * T] = oc.transpose(1, 0, 2).reshape(T, H)
        ent_total += np.float64(res.results[c]["ent"].sum(dtype=np.float64))
    loss = np.float32(-(ent_total / NTOK))
    return out.reshape(B, S, H), loss


def kernel(**inputs):
    return run_full(inputs, trace=False)


# revision 14
# speedup vs baseline: 1.3838x; 1.0865x over previous
# MoE layer (8 experts, top-2 routing) on 8 trn2 NeuronCores.
# Sharding: token-parallel. Each core processes 512 of the 4096 tokens:
# full router + all 8 experts (dense eval, matching the reference) on its
# token slice. No collectives; host concatenates the per-core outputs.
#
# Layouts: activations are feature-major [feat_part, tok_free] through the
# MLPs (contraction dim always on partitions -> no transposes); the last
# expert matmul (fc2) swaps lhsT/rhs so its output lands token-major
# [tok_part, feat_free], where layernorm + the prob-weighted combine only
# need per-partition scalars. The input-independent attr projection and the
# ap-half of gate_w1 are folded into host-precomputed biases.
import numpy as np
import ml_dtypes
from contextlib import ExitStack

import concourse.bass as bass
import concourse.bacc as bacc
import concourse.mybir as mybir
import concourse.tile as tile
from concourse.bass import ts
from concourse.bass_utils import run_bass_kernel_spmd

F32 = mybir.dt.float32
F32R = mybir.dt.float32r
BF16 = mybir.dt.bfloat16
ROUTER_F32R = True   # run the two big router matmul layers as float32r
RDT = F32R if ROUTER_F32R else F32
AF = mybir.ActivationFunctionType
ALU = mybir.AluOpType
AXX = mybir.AxisListType.X

H, E, TD, NT, TOPK, I = 1024, 8, 64, 4, 2, 2048
A = TD * NT
B, S = 4, 1024
NTOK = B * S
NCORE = 8
T = NTOK // NCORE          # tokens per core
P = 128
TCH = T // P               # token chunks per core
EXPERT_NP = ml_dtypes.bfloat16
EXPERT_DT = BF16

_PROG_CACHE = {}
LAST_RESULTS = None


def _build_program(ln_trivial=False):
    nc = bacc.Bacc("TRN2", target_bir_lowering=False, debug=False,
                   enable_asserts=False, num_devices=NCORE)

    def din(name, shape, dt=F32):
        return nc.dram_tensor(name, list(shape), dt, kind="ExternalInput").ap()

    # per-core inputs
    x_fm_bf_d = din("x_fm_bf", [P, H // P, T], EXPERT_DT)
    x_fm_d = din("x_fm", [P, H // P, T], RDT)
    x_tok_d = din("x_tok", [P, TCH, H])
    te_rhs_d = din("te_rhs", [P, A // P, T], RDT)
    te_attr_d = din("te_attr", [TD, NT, T])
    # replicated router weights
    R1_d = din("R1", [P, 4 * H // P, 10, P], RDT)       # r_in_w^T packed (ki, oc, ko, oi)
    rib_d = din("rib", [P, 4 * H // P])
    R2_d = din("R2", [P, H // P, 32, P], RDT)
    rmb_d = din("rmb", [P, H // P])
    RW_d = din("RW", [P, H // P, E])
    rbb_d = din("rbb", [P, E])
    RA_d = din("RA", [TD, E])
    # replicated expert weights (stacked on E)
    W1_d = din("W1", [E, P, H // P, I], EXPERT_DT)
    C1_d = din("C1", [P, E, I // P])
    G2_d = din("G2", [E, P, I // P, H], EXPERT_DT)
    C2_d = din("C2", [P, E, H // P])
    F1_d = din("F1", [E, P, H // P, I], EXPERT_DT)
    B1_d = din("B1", [P, E, I // P])
    F2_d = din("F2", [E, P, I // P, H], EXPERT_DT)
    FB2_d = din("FB2", [E, P, H])
    LG_d = din("LG", [E, P, H])
    LB_d = din("LB", [E, P, H])
    AP_d = din("APc", [P, E, H // P])

    out_d = nc.dram_tensor("out", [P, TCH, H], F32, kind="ExternalOutput").ap()
    ent_d = nc.dram_tensor("ent", [P, TCH], F32, kind="ExternalOutput").ap()

    with tile.TileContext(nc) as tc:
        with ExitStack() as top:
            const = top.enter_context(tc.tile_pool(name="const", bufs=1))
            ps_mm = top.enter_context(tc.tile_pool(name="ps_mm", bufs=2, space="PSUM"))

            x_fm_bf = const.tile([P, H // P, T], EXPERT_DT)
            nc.sync.dma_start(x_fm_bf[:], x_fm_bf_d)
            x_tok = const.tile([P, TCH, H], F32)
            nc.sync.dma_start(x_tok[:], x_tok_d)
            c1_sb = const.tile([P, E, I // P], F32)
            nc.sync.dma_start(c1_sb[:], C1_d)
            c2_sb = const.tile([P, E, H // P], F32)
            nc.sync.dma_start(c2_sb[:], C2_d)
            b1_sb = const.tile([P, E, I // P], F32)
            nc.sync.dma_start(b1_sb[:], B1_d)
            ap_sb = const.tile([P, E, H // P], F32)
            nc.sync.dma_start(ap_sb[:], AP_d)
            probs = const.tile([P, TCH, E], F32)
            ent = const.tile([P, TCH], F32)
            eps8 = const.tile([P, 1], F32)
            nc.vector.memset(eps8[:], 1e-8)

            # ---------------- router (fp32) ----------------
            with ExitStack() as rs:
                rpool = rs.enter_context(tc.tile_pool(name="rconst", bufs=1))
                r1p = rs.enter_context(tc.tile_pool(name="r1", bufs=3))
                r2p = rs.enter_context(tc.tile_pool(name="r2", bufs=2))
                h1p = rs.enter_context(tc.tile_pool(name="h1", bufs=1))
                ps_sm = rs.enter_context(tc.tile_pool(name="ps_sm", bufs=2, space="PSUM"))
                smt = rs.enter_context(tc.tile_pool(name="smt", bufs=4))

                x_fm = rpool.tile([P, H // P, T], RDT)
                nc.sync.dma_start(x_fm[:], x_fm_d)
                te_rhs = rpool.tile([P, A // P, T], RDT)
                nc.sync.dma_start(te_rhs[:], te_rhs_d)
                te_attr = rpool.tile([TD, NT, T], F32)
                nc.sync.dma_start(te_attr[:], te_attr_d)
                ra_sb = rpool.tile([TD, E], F32)
                nc.sync.dma_start(ra_sb[:], RA_d)
                rib_sb = rpool.tile([P, 4 * H // P], F32)
                nc.sync.dma_start(rib_sb[:], rib_d)
                rmb_sb = rpool.tile([P, H // P], F32)
                nc.sync.dma_start(rmb_sb[:], rmb_d)
                rw_sb = rpool.tile([P, H // P, E], F32)
                nc.sync.dma_start(rw_sb[:], RW_d)
                rbb_sb = rpool.tile([P, E], F32)
                nc.sync.dma_start(rbb_sb[:], rbb_d)

                h1 = h1p.tile([P, 4 * H // P, T], RDT)
                h2 = rpool.tile([P, H // P, T], F32)

                rhs_list = [x_fm[:, i] for i in range(H // P)] + \
                           [te_rhs[:, i] for i in range(A // P)]
                # iv1 = relu([x, te] @ r_in_w.T + r_in_b)   [4H, T]
                for oc in range(4 * H // P):
                    w = r1p.tile([P, 10, P], RDT, tag="r1w")
                    nc.sync.dma_start(w[:], R1_d[:, oc])
                    ps = ps_mm.tile([P, T], F32, tag="mm")
                    for kt in range(10):
                        nc.tensor.matmul(ps[:], w[:, kt], rhs_list[kt],
                                         start=(kt == 0), stop=(kt == 9))
                    nc.scalar.activation(h1[:, oc], ps[:], AF.Relu,
                                         bias=rib_sb[:, oc:oc + 1])
                # iv2 = relu(iv1 @ r_mid_w.T + r_mid_b)   [H, T]
                for oc in range(H // P):
                    w = r2p.tile([P, 32, P], RDT, tag="r2w")
                    nc.sync.dma_start(w[:], R2_d[:, oc])
                    ps = ps_mm.tile([P, T], F32, tag="mm")
                    for kt in range(32):
                        nc.tensor.matmul(ps[:], w[:, kt], h1[:, kt],
                                         start=(kt == 0), stop=(kt == 31))
                    nc.scalar.activation(h2[:, oc], ps[:], AF.Relu,
                                         bias=rmb_sb[:, oc:oc + 1])
                # logits / softmax / attr softmax / top-2 per token chunk
                for tcx in range(TCH):
                    ps = ps_sm.tile([P, E], F32, tag="sm")
                    for kt in range(H // P):
                        nc.tensor.matmul(ps[:], h2[:, kt, ts(tcx, P)], rw_sb[:, kt],
                                         start=(kt == 0), stop=(kt == H // P - 1))
                    lg = smt.tile([P, E], F32, tag="s8")
                    nc.vector.tensor_add(lg[:], ps[:], rbb_sb[:])
                    nmax = smt.tile([P, 1], F32, tag="s1")
                    nc.vector.reduce_max(nmax[:], lg[:], axis=AXX, negate=True)
                    sume = smt.tile([P, 1], F32, tag="s1")
                    ep = smt.tile([P, E], F32, tag="s8")
                    nc.scalar.activation(ep[:], lg[:], AF.Exp, bias=nmax[:],
                                         accum_out=sume[:])
                    rec = smt.tile([P, 1], F32, tag="s1")
                    nc.vector.reciprocal(rec[:], sume[:])
                    nc.vector.tensor_scalar_mul(ep[:], ep[:], rec[:])
                    asum = smt.tile([P, E], F32, tag="s8")
                    for n in range(NT):
                        psa = ps_sm.tile([P, E], F32, tag="sm")
                        nc.tensor.matmul(psa[:], te_attr[:, n, ts(tcx, P)], ra_sb[:],
                                         start=True, stop=True)
                        nma = smt.tile([P, 1], F32, tag="s1")
                        nc.vector.reduce_max(nma[:], psa[:], axis=AXX, negate=True)
                        suma = smt.tile([P, 1], F32, tag="s1")
                        an = smt.tile([P, E], F32, tag="s8")
                        nc.scalar.activation(an[:], psa[:], AF.Exp, bias=nma[:],
                                             accum_out=suma[:])
                        reca = smt.tile([P, 1], F32, tag="s1")
                        nc.vector.reciprocal(reca[:], suma[:])
                        if n == 0:
                            nc.vector.tensor_scalar_mul(asum[:], an[:], reca[:])
                        else:
                            tmp8 = smt.tile([P, E], F32, tag="s8")
                            nc.vector.tensor_scalar_mul(tmp8[:], an[:], reca[:])
                            nc.vector.tensor_add(asum[:], asum[:], tmp8[:])
                    pf = smt.tile([P, E], F32, tag="s8")
                    nc.vector.tensor_mul(pf[:], ep[:], asum[:])
                    nc.vector.tensor_scalar_mul(pf[:], pf[:], 1.0 / NT)
                    # top-2 mask (values >= 2nd max)
                    m1 = smt.tile([P, 1], F32, tag="s1")
                    nc.vector.reduce_max(m1[:], pf[:], axis=AXX)
                    eq = smt.tile([P, E], F32, tag="s8")
                    nc.vector.tensor_single_scalar(eq[:], pf[:], m1[:], ALU.is_ge)
                    nc.vector.tensor_scalar_mul(eq[:], eq[:], -1e30)
                    p2t = smt.tile([P, E], F32, tag="s8")
                    nc.vector.tensor_add(p2t[:], pf[:], eq[:])
                    m2 = smt.tile([P, 1], F32, tag="s1")
                    nc.vector.reduce_max(m2[:], p2t[:], axis=AXX)
                    msk = smt.tile([P, E], F32, tag="s8")
                    nc.vector.tensor_single_scalar(msk[:], pf[:], m2[:], ALU.is_ge)
                    nc.vector.tensor_mul(probs[:, tcx], pf[:], msk[:])
                    # entropy partial: sum_e p*log(p+1e-8)
                    l8 = smt.tile([P, E], F32, tag="s8")
                    nc.scalar.activation(l8[:], probs[:, tcx], AF.Ln, bias=eps8[:])
                    scr = smt.tile([P, E], F32, tag="s8")
                    nc.vector.tensor_mul(scr[:], probs[:, tcx], l8[:])
                    nc.vector.reduce_sum(ent[:, tcx:tcx + 1], scr[:], axis=AXX)
                nc.sync.dma_start(ent_d, ent[:])

            # ---------------- experts (bf16) ----------------
            with ExitStack() as es:
                wp = es.enter_context(tc.tile_pool(name="w", bufs=6))
                zp = es.enter_context(tc.tile_pool(name="z", bufs=2))
                gp = es.enter_context(tc.tile_pool(name="g", bufs=3))
                hp = es.enter_context(tc.tile_pool(name="h", bufs=2))
                lnp = es.enter_context(tc.tile_pool(name="ln", bufs=2))
                ept = es.enter_context(tc.tile_pool(name="ept", bufs=3))
                sp = es.enter_context(tc.tile_pool(name="sp", bufs=6))
                ps_out = es.enter_context(tc.tile_pool(name="ps_out", bufs=4, space="PSUM"))

                for e in range(E):
                    # z1 = relu(x @ W1x.T + c1)    [I, T]
                    w1h = []
                    for hf in range(2):
                        w = wp.tile([P, H // P, I // 2], EXPERT_DT, tag="w")
                        nc.sync.dma_start(w[:], W1_d[e][:, :, ts(hf, I // 2)])
                        w1h.append(w)
                    z1 = zp.tile([P, I // P, T], EXPERT_DT, tag="z")
                    for oc in range(I // P):
                        ps = ps_mm.tile([P, T], F32, tag="mm")
                        for kt in range(H // P):
                            nc.tensor.matmul(ps[:], w1h[oc // 8][:, kt, ts(oc % 8, P)],
                                             x_fm_bf[:, kt],
                                             start=(kt == 0), stop=(kt == H // P - 1))
                        nc.scalar.activation(z1[:, oc], ps[:], AF.Relu,
                                             bias=c1_sb[:, e, oc:oc + 1])
                    # gate = sigmoid(z1 @ gw2.T + gb2); h = gate*(x-ap)+ap  [H, T]
                    g2h = []
                    for hf in range(2):
                        w = wp.tile([P, I // P, H // 2], EXPERT_DT, tag="w")
                        nc.sync.dma_start(w[:], G2_d[e][:, :, ts(hf, H // 2)])
                        g2h.append(w)
                    hb = hp.tile([P, H // P, T], EXPERT_DT, tag="h")
                    for oc in range(H // P):
                        ps = ps_mm.tile([P, T], F32, tag="mm")
                        for kt in range(I // P):
                            nc.tensor.matmul(ps[:], g2h[oc // 4][:, kt, ts(oc % 4, P)],
                                             z1[:, kt],
                                             start=(kt == 0), stop=(kt == I // P - 1))
                        gt = gp.tile([P, T], F32, tag="gt")
                        nc.scalar.activation(gt[:], ps[:], AF.Sigmoid,
                                             bias=c2_sb[:, e, oc:oc + 1])
                        xm = gp.tile([P, T], F32, tag="xm")
                        nc.vector.tensor_single_scalar(xm[:], x_fm_bf[:, oc],
                                                       ap_sb[:, e, oc:oc + 1], ALU.subtract)
                        nc.vector.tensor_mul(xm[:], xm[:], gt[:])
                        nc.vector.tensor_single_scalar(hb[:, oc], xm[:],
                                                       ap_sb[:, e, oc:oc + 1], ALU.add)
                    # z2 = relu(h @ fw1.T + fb1)   [I, T]
                    f1h = []
                    for hf in range(2):
                        w = wp.tile([P, H // P, I // 2], EXPERT_DT, tag="w")
                        nc.sync.dma_start(w[:], F1_d[e][:, :, ts(hf, I // 2)])
                        f1h.append(w)
                    z2 = zp.tile([P, I // P, T], EXPERT_DT, tag="z")
                    for oc in range(I // P):
                        ps = ps_mm.tile([P, T], F32, tag="mm")
                        for kt in range(H // P):
                            nc.tensor.matmul(ps[:], f1h[oc // 8][:, kt, ts(oc % 8, P)],
                                             hb[:, kt],
                                             start=(kt == 0), stop=(kt == H // P - 1))
                        nc.scalar.activation(z2[:, oc], ps[:], AF.Relu,
                                             bias=b1_sb[:, e, oc:oc + 1])
                    # o = z2 @ fw2.T + fb2 (token-major out), r = o + x,
                    # y = LN(r)*g+b, out += prob_e * y
                    f2h = []
                    for hf in range(2):
                        w = wp.tile([P, I // P, H // 2], EXPERT_DT, tag="w")
                        nc.sync.dma_start(w[:], F2_d[e][:, :, ts(hf, H // 2)])
                        f2h.append(w)
                    fb2_sb = lnp.tile([P, H], F32, tag="fb2")
                    nc.sync.dma_start(fb2_sb[:], FB2_d[e])
                    if not ln_trivial:
                        lg_sb = lnp.tile([P, H], F32, tag="lg")
                        nc.sync.dma_start(lg_sb[:], LG_d[e])
                        lb_sb = lnp.tile([P, H], F32, tag="lb")
                        nc.sync.dma_start(lb_sb[:], LB_d[e])
                    for tcx in range(TCH):
                        t1 = ept.tile([P, H], F32, tag="t")
                        for hh in range(H // 512):
                            pso = ps_out.tile([P, 512], F32, tag="out")
                            for kt in range(I // P):
                                nc.tensor.matmul(pso[:],
                                                 z2[:, kt, ts(tcx, P)],
                                                 f2h[hh][:, kt],
                                                 start=(kt == 0), stop=(kt == I // P - 1))
                            nc.vector.tensor_add(t1[:, ts(hh, 512)], pso[:],
                                                 fb2_sb[:, ts(hh, 512)])
                        r = ept.tile([P, H], F32, tag="t")
                        s1 = sp.tile([P, 1], F32, tag="sp")
                        nc.vector.tensor_add(r[:], t1[:], x_tok[:, tcx])
                        nc.vector.reduce_sum(s1[:], r[:], axis=AXX)
                        s2 = sp.tile([P, 1], F32, tag="sp")
                        nc.scalar.activation(t1[:], r[:], AF.Square, accum_out=s2[:])
                        mu = sp.tile([P, 1], F32, tag="sp")
                        nc.vector.tensor_scalar_mul(mu[:], s1[:], 1.0 / H)
                        nm = sp.tile([P, 1], F32, tag="sp")
                        nc.vector.tensor_mul(nm[:], mu[:], mu[:])
                        nc.vector.tensor_scalar(nm[:], nm[:], -1.0, 1e-5, ALU.mult, ALU.add)
                        std = sp.tile([P, 1], F32, tag="sp")
                        nc.scalar.activation(std[:], s2[:], AF.Sqrt, bias=nm[:],
                                             scale=1.0 / H)
                        rstd = sp.tile([P, 1], F32, tag="sp")
                        nc.vector.reciprocal(rstd[:], std[:])
                        yn = ept.tile([P, H], F32, tag="t")
                        nc.vector.tensor_scalar(yn[:], r[:], mu[:], rstd[:],
                                                ALU.subtract, ALU.mult)
                        if not ln_trivial:
                            nc.vector.tensor_mul(yn[:], yn[:], lg_sb[:])
                            nc.vector.tensor_add(yn[:], yn[:], lb_sb[:])
                        wy = ept.tile([P, H], F32, tag="t")
                        nc.vector.tensor_single_scalar(wy[:], yn[:],
                                                       probs[:, tcx, e:e + 1], ALU.mult)
                        if e == 0:
                            nc.sync.dma_start(out_d[:, tcx], wy[:])
                        else:
                            nc.gpsimd.dma_start(out_d[:, tcx], wy[:], accum_op=ALU.add)

    nc.compile()
    return nc


def _pack_weights(i):
    f32 = np.float32
    gw1 = np.asarray(i["gate_w1"], f32)
    gb1 = np.asarray(i["gate_b1"], f32)
    gw2 = np.asarray(i["gate_w2"], f32)
    gb2 = np.asarray(i["gate_b2"], f32)
    fw1 = np.asarray(i["fc_w1"], f32)
    fb1 = np.asarray(i["fc_b1"], f32)
    fw2 = np.asarray(i["fc_w2"], f32)
    fb2 = np.asarray(i["fc_b2"], f32)
    ln_g = np.asarray(i["ln_g"], f32)
    ln_b = np.asarray(i["ln_b"], f32)
    ae = np.asarray(i["attr_emb"], f32)
    apw = np.asarray(i["attr_proj_w"], f32)
    apb = np.asarray(i["attr_proj_b"], f32)
    r_attr = np.asarray(i["r_attr"], f32)
    r_in_w = np.asarray(i["r_in_w"], f32)
    r_in_b = np.asarray(i["r_in_b"], f32)
    r_mid_w = np.asarray(i["r_mid_w"], f32)
    r_mid_b = np.asarray(i["r_mid_b"], f32)
    r_w = np.asarray(i["r_w"], f32)
    r_b = np.asarray(i["r_b"], f32)

    C = np.ascontiguousarray
    # input-independent precomputation
    ap_vec = np.einsum("ea,eha->eh", ae[:, 0], apw) + apb           # [E,H]
    c1 = np.einsum("eh,eih->ei", ap_vec, gw1[:, :, H:]) + gb1       # [E,I]
    W1x = gw1[:, :, :H]                                             # [E,I,H]

    def pack_lhsT(wT):  # [K,O] -> [P, K//P, O]
        K, O = wT.shape
        return C(wT.reshape(K // P, P, O).transpose(1, 0, 2))

    def chunks(v, nch):  # [E, nch*P] -> [P, E, nch]
        return C(np.stack([v[e].reshape(nch, P).T for e in range(E)], axis=1))

    d = {}
    r_in_wT = C(r_in_w.T)     # [1280, 4096]
    d["R1"] = C(r_in_wT.reshape(10, P, 32, P).transpose(1, 2, 0, 3))
    d["rib"] = C(r_in_b.reshape(32, P).T)
    r_mid_wT = C(r_mid_w.T)   # [4096, 1024]
    d["R2"] = C(r_mid_wT.reshape(32, P, 8, P).transpose(1, 2, 0, 3))
    d["rmb"] = C(r_mid_b.reshape(8, P).T)
    d["RW"] = C(r_w.T.reshape(8, P, E).transpose(1, 0, 2))
    d["rbb"] = C(np.tile(r_b, (P, 1)))
    d["RA"] = C(r_attr)
    d["W1"] = np.stack([pack_lhsT(C(W1x[e].T)) for e in range(E)]).astype(EXPERT_NP)
    d["C1"] = chunks(c1, I // P)
    d["G2"] = np.stack([pack_lhsT(C(gw2[e].T)) for e in range(E)]).astype(EXPERT_NP)
    d["C2"] = chunks(gb2, H // P)
    d["F1"] = np.stack([pack_lhsT(C(fw1[e].T)) for e in range(E)]).astype(EXPERT_NP)
    d["B1"] = chunks(fb1, I // P)
    d["F2"] = np.stack([pack_lhsT(C(fw2[e].T)) for e in range(E)]).astype(EXPERT_NP)
    d["FB2"] = C(np.tile(fb2[:, None, :], (1, P, 1)))
    d["LG"] = C(np.tile(ln_g[:, None, :], (1, P, 1)))
    d["LB"] = C(np.tile(ln_b[:, None, :], (1, P, 1)))
    d["APc"] = chunks(ap_vec, H // P)
    return d


def _pack_core_inputs(x, te, c):
    C = np.ascontiguousarray
    sl = slice(c * T, (c + 1) * T)
    xs = x.reshape(NTOK, H)[sl]                    # [T,H]
    te_s = te.reshape(NTOK, NT, TD)[sl]            # [T,NT,TD]
    xT = C(xs.T)                                   # [H,T]
    x_fm = C(xT.reshape(H // P, P, T).transpose(1, 0, 2))
    d = {
        "x_fm": x_fm,
        "x_fm_bf": x_fm.astype(EXPERT_NP),
        "x_tok": C(xs.reshape(TCH, P, H).transpose(1, 0, 2)),
        "te_rhs": C(te_s.reshape(T, A).T.reshape(A // P, P, T).transpose(1, 0, 2)),
        "te_attr": C(te_s.transpose(2, 1, 0)),
    }
    return d


def run_full(inputs, trace=False):
    global LAST_RESULTS
    x = np.asarray(inputs["x"], np.float32)
    te = np.asarray(inputs["task_embeddings"], np.float32)

    ln_trivial = bool(
        np.all(np.asarray(inputs["ln_g"], np.float32) == 1.0)
        and np.all(np.asarray(inputs["ln_b"], np.float32) == 0.0))
    key = ("prog", ln_trivial)
    if key not in _PROG_CACHE:
        _PROG_CACHE[key] = _build_program(ln_trivial=ln_trivial)
    nc = _PROG_CACHE[key]

    wmap = _pack_weights(inputs)
    in_maps = []
    for c in range(NCORE):
        m = dict(wmap)
        m.update(_pack_core_inputs(x, te, c))
        in_maps.append(m)

    res = run_bass_kernel_spmd(nc, in_maps, core_ids=list(range(NCORE)), trace=trace)
    LAST_RESULTS = res

    out = np.empty((NTOK, H), np.float32)
    ent_total = np.float64(0.0)
    for c in range(NCORE):
        oc = res.results[c]["out"]                 # [P, TCH, H]
        out[c * T:(c + 1) * T] = oc.transpose(1, 0, 2).reshape(T, H)
        ent_total += np.float64(res.results[c]["ent"].sum(dtype=np.float64))
    loss = np.float32(-(ent_total / NTOK))
    return out.reshape(B, S, H), loss


def kernel(**inputs):
    return run_full(inputs, trace=False)


# revision 16
# speedup vs baseline: 1.3874x; 1.0026x over previous
# MoE layer (8 experts, top-2 routing) on 8 trn2 NeuronCores.
# Sharding: token-parallel. Each core processes 512 of the 4096 tokens:
# full router + all 8 experts (dense eval, matching the reference) on its
# token slice. No collectives; host concatenates the per-core outputs.
#
# Layouts: activations are feature-major [feat_part, tok_free] through the
# MLPs (contraction dim always on partitions -> no transposes); the last
# expert matmul (fc2) swaps lhsT/rhs so its output lands token-major
# [tok_part, feat_free], where layernorm + the prob-weighted combine only
# need per-partition scalars. The input-independent attr projection and the
# ap-half of gate_w1 are folded into host-precomputed biases.
import numpy as np
import ml_dtypes
from contextlib import ExitStack

import concourse.bass as bass
import concourse.bacc as bacc
import concourse.mybir as mybir
import concourse.tile as tile
from concourse.bass import ts
from concourse.bass_utils import run_bass_kernel_spmd

F32 = mybir.dt.float32
F32R = mybir.dt.float32r
BF16 = mybir.dt.bfloat16
ROUTER_F32R = True   # run the two big router matmul layers as float32r
RDT = F32R if ROUTER_F32R else F32
AF = mybir.ActivationFunctionType
ALU = mybir.AluOpType
AXX = mybir.AxisListType.X

H, E, TD, NT, TOPK, I = 1024, 8, 64, 4, 2, 2048
A = TD * NT
B, S = 4, 1024
NTOK = B * S
NCORE = 8
T = NTOK // NCORE          # tokens per core
P = 128
TCH = T // P               # token chunks per core
EXPERT_NP = ml_dtypes.bfloat16
EXPERT_DT = BF16

_PROG_CACHE = {}
LAST_RESULTS = None


def _build_program(ln_trivial=False):
    nc = bacc.Bacc("TRN2", target_bir_lowering=False, debug=False,
                   enable_asserts=False, num_devices=NCORE)

    def din(name, shape, dt=F32):
        return nc.dram_tensor(name, list(shape), dt, kind="ExternalInput").ap()

    # per-core inputs
    x_fm_bf_d = din("x_fm_bf", [P, H // P, T], EXPERT_DT)
    x_fm_d = din("x_fm", [P, H // P, T], RDT)
    x_tok_d = din("x_tok", [P, TCH, H])
    te_rhs_d = din("te_rhs", [P, A // P, T], RDT)
    te_attr_d = din("te_attr", [TD, NT, T])
    # replicated router weights
    R1_d = din("R1", [P, 4 * H // P, 10, P], RDT)       # r_in_w^T packed (ki, oc, ko, oi)
    rib_d = din("rib", [P, 4 * H // P])
    R2_d = din("R2", [P, H // P, 32, P], RDT)
    rmb_d = din("rmb", [P, H // P])
    RW_d = din("RW", [P, H // P, E])
    rbb_d = din("rbb", [P, E])
    RA_d = din("RA", [TD, E])
    # replicated expert weights (stacked on E)
    W1_d = din("W1", [E, P, H // P, I], EXPERT_DT)
    C1_d = din("C1", [P, E, I // P])
    G2_d = din("G2", [E, P, I // P, H], EXPERT_DT)
    C2_d = din("C2", [P, E, H // P])
    F1_d = din("F1", [E, P, H // P, I], EXPERT_DT)
    B1_d = din("B1", [P, E, I // P])
    F2_d = din("F2", [E, P, I // P, H], EXPERT_DT)
    FB2_d = din("FB2", [E, P, H])
    LG_d = din("LG", [E, P, H])
    LB_d = din("LB", [E, P, H])
    AP_d = din("APc", [P, E, H // P])

    out_d = nc.dram_tensor("out", [P, TCH, H], F32, kind="ExternalOutput").ap()
    ent_d = nc.dram_tensor("ent", [P, TCH], F32, kind="ExternalOutput").ap()

    with tile.TileContext(nc) as tc:
        with ExitStack() as top:
            const = top.enter_context(tc.tile_pool(name="const", bufs=1))
            ps_mm = top.enter_context(tc.tile_pool(name="ps_mm", bufs=2, space="PSUM"))

            x_fm_bf = const.tile([P, H // P, T], EXPERT_DT)
            for i in range(H // P):
                nc.sync.dma_start(x_fm_bf[:, i], x_fm_bf_d[:, i])
            x_tok = const.tile([P, TCH, H], F32)
            for i in range(TCH):
                nc.sync.dma_start(x_tok[:, i], x_tok_d[:, i])
            c1_sb = const.tile([P, E, I // P], F32)
            nc.sync.dma_start(c1_sb[:], C1_d)
            c2_sb = const.tile([P, E, H // P], F32)
            nc.sync.dma_start(c2_sb[:], C2_d)
            b1_sb = const.tile([P, E, I // P], F32)
            nc.sync.dma_start(b1_sb[:], B1_d)
            ap_sb = const.tile([P, E, H // P], F32)
            nc.sync.dma_start(ap_sb[:], AP_d)
            probs = const.tile([P, TCH, E], F32)
            ent = const.tile([P, TCH], F32)
            eps8 = const.tile([P, 1], F32)
            nc.vector.memset(eps8[:], 1e-8)

            # ---------------- router (fp32) ----------------
            with ExitStack() as rs:
                rpool = rs.enter_context(tc.tile_pool(name="rconst", bufs=1))
                r1p = rs.enter_context(tc.tile_pool(name="r1", bufs=3))
                r2p = rs.enter_context(tc.tile_pool(name="r2", bufs=2))
                h1p = rs.enter_context(tc.tile_pool(name="h1", bufs=1))
                ps_sm = rs.enter_context(tc.tile_pool(name="ps_sm", bufs=2, space="PSUM"))
                smt = rs.enter_context(tc.tile_pool(name="smt", bufs=4))

                x_fm = rpool.tile([P, H // P, T], RDT)
                for i in range(H // P):
                    nc.sync.dma_start(x_fm[:, i], x_fm_d[:, i])
                te_rhs = rpool.tile([P, A // P, T], RDT)
                for i in range(A // P):
                    nc.sync.dma_start(te_rhs[:, i], te_rhs_d[:, i])
                te_attr = rpool.tile([TD, NT, T], F32)
                nc.sync.dma_start(te_attr[:], te_attr_d)
                ra_sb = rpool.tile([TD, E], F32)
                nc.sync.dma_start(ra_sb[:], RA_d)
                rib_sb = rpool.tile([P, 4 * H // P], F32)
                nc.sync.dma_start(rib_sb[:], rib_d)
                rmb_sb = rpool.tile([P, H // P], F32)
                nc.sync.dma_start(rmb_sb[:], rmb_d)
                rw_sb = rpool.tile([P, H // P, E], F32)
                nc.sync.dma_start(rw_sb[:], RW_d)
                rbb_sb = rpool.tile([P, E], F32)
                nc.sync.dma_start(rbb_sb[:], rbb_d)

                h1 = h1p.tile([P, 4 * H // P, T], RDT)
                h2 = rpool.tile([P, H // P, T], F32)

                rhs_list = [x_fm[:, i] for i in range(H // P)] + \
                           [te_rhs[:, i] for i in range(A // P)]
                # iv1 = relu([x, te] @ r_in_w.T + r_in_b)   [4H, T]
                for oc in range(4 * H // P):
                    w = r1p.tile([P, 10, P], RDT, tag="r1w")
                    nc.sync.dma_start(w[:], R1_d[:, oc])
                    ps = ps_mm.tile([P, T], F32, tag="mm")
                    for kt in range(10):
                        nc.tensor.matmul(ps[:], w[:, kt], rhs_list[kt],
                                         start=(kt == 0), stop=(kt == 9))
                    nc.scalar.activation(h1[:, oc], ps[:], AF.Relu,
                                         bias=rib_sb[:, oc:oc + 1])
                # iv2 = relu(iv1 @ r_mid_w.T + r_mid_b)   [H, T]
                for oc in range(H // P):
                    w = r2p.tile([P, 32, P], RDT, tag="r2w")
                    nc.sync.dma_start(w[:], R2_d[:, oc])
                    ps = ps_mm.tile([P, T], F32, tag="mm")
                    for kt in range(32):
                        nc.tensor.matmul(ps[:], w[:, kt], h1[:, kt],
                                         start=(kt == 0), stop=(kt == 31))
                    nc.scalar.activation(h2[:, oc], ps[:], AF.Relu,
                                         bias=rmb_sb[:, oc:oc + 1])
                # logits / softmax / attr softmax / top-2 per token chunk
                for tcx in range(TCH):
                    ps = ps_sm.tile([P, E], F32, tag="sm")
                    for kt in range(H // P):
                        nc.tensor.matmul(ps[:], h2[:, kt, ts(tcx, P)], rw_sb[:, kt],
                                         start=(kt == 0), stop=(kt == H // P - 1))
                    lg = smt.tile([P, E], F32, tag="s8")
                    nc.vector.tensor_add(lg[:], ps[:], rbb_sb[:])
                    nmax = smt.tile([P, 1], F32, tag="s1")
                    nc.vector.reduce_max(nmax[:], lg[:], axis=AXX, negate=True)
                    sume = smt.tile([P, 1], F32, tag="s1")
                    ep = smt.tile([P, E], F32, tag="s8")
                    nc.scalar.activation(ep[:], lg[:], AF.Exp, bias=nmax[:],
                                         accum_out=sume[:])
                    rec = smt.tile([P, 1], F32, tag="s1")
                    nc.vector.reciprocal(rec[:], sume[:])
                    nc.vector.tensor_scalar_mul(ep[:], ep[:], rec[:])
                    asum = smt.tile([P, E], F32, tag="s8")
                    for n in range(NT):
                        psa = ps_sm.tile([P, E], F32, tag="sm")
                        nc.tensor.matmul(psa[:], te_attr[:, n, ts(tcx, P)], ra_sb[:],
                                         start=True, stop=True)
                        nma = smt.tile([P, 1], F32, tag="s1")
                        nc.vector.reduce_max(nma[:], psa[:], axis=AXX, negate=True)
                        suma = smt.tile([P, 1], F32, tag="s1")
                        an = smt.tile([P, E], F32, tag="s8")
                        nc.scalar.activation(an[:], psa[:], AF.Exp, bias=nma[:],
                                             accum_out=suma[:])
                        reca = smt.tile([P, 1], F32, tag="s1")
                        nc.vector.reciprocal(reca[:], suma[:])
                        if n == 0:
                            nc.vector.tensor_scalar_mul(asum[:], an[:], reca[:])
                        else:
                            tmp8 = smt.tile([P, E], F32, tag="s8")
                            nc.vector.tensor_scalar_mul(tmp8[:], an[:], reca[:])
                            nc.vector.tensor_add(asum[:], asum[:], tmp8[:])
                    pf = smt.tile([P, E], F32, tag="s8")
                    nc.vector.tensor_mul(pf[:], ep[:], asum[:])
                    nc.vector.tensor_scalar_mul(pf[:], pf[:], 1.0 / NT)
                    # top-2 mask (values >= 2nd max)
                    m1 = smt.tile([P, 1], F32, tag="s1")
                    nc.vector.reduce_max(m1[:], pf[:], axis=AXX)
                    eq = smt.tile([P, E], F32, tag="s8")
                    nc.vector.tensor_single_scalar(eq[:], pf[:], m1[:], ALU.is_ge)
                    nc.vector.tensor_scalar_mul(eq[:], eq[:], -1e30)
                    p2t = smt.tile([P, E], F32, tag="s8")
                    nc.vector.tensor_add(p2t[:], pf[:], eq[:])
                    m2 = smt.tile([P, 1], F32, tag="s1")
                    nc.vector.reduce_max(m2[:], p2t[:], axis=AXX)
                    msk = smt.tile([P, E], F32, tag="s8")
                    nc.vector.tensor_single_scalar(msk[:], pf[:], m2[:], ALU.is_ge)
                    nc.vector.tensor_mul(probs[:, tcx], pf[:], msk[:])
                    # entropy partial: sum_e p*log(p+1e-8)
                    l8 = smt.tile([P, E], F32, tag="s8")
                    nc.scalar.activation(l8[:], probs[:, tcx], AF.Ln, bias=eps8[:])
                    scr = smt.tile([P, E], F32, tag="s8")
                    nc.vector.tensor_mul(scr[:], probs[:, tcx], l8[:])
                    nc.vector.reduce_sum(ent[:, tcx:tcx + 1], scr[:], axis=AXX)
                nc.sync.dma_start(ent_d, ent[:])

            # ---------------- experts (bf16) ----------------
            with ExitStack() as es:
                wp = es.enter_context(tc.tile_pool(name="w", bufs=6 if ln_trivial else 5))
                zp = es.enter_context(tc.tile_pool(name="z", bufs=2))
                gp = es.enter_context(tc.tile_pool(name="g", bufs=3))
                hp = es.enter_context(tc.tile_pool(name="h", bufs=2))
                lnp = es.enter_context(tc.tile_pool(name="ln", bufs=2))
                ept = es.enter_context(tc.tile_pool(name="ept", bufs=3))
                sp = es.enter_context(tc.tile_pool(name="sp", bufs=6))
                ps_out = es.enter_context(tc.tile_pool(name="ps_out", bufs=4, space="PSUM"))

                for e in range(E):
                    # z1 = relu(x @ W1x.T + c1)    [I, T]
                    w1h = []
                    for hf in range(2):
                        w = wp.tile([P, H // P, I // 2], EXPERT_DT, tag="w")
                        nc.sync.dma_start(w[:], W1_d[e][:, :, ts(hf, I // 2)])
                        w1h.append(w)
                    z1 = zp.tile([P, I // P, T], EXPERT_DT, tag="z")
                    for oc in range(I // P):
                        ps = ps_mm.tile([P, T], F32, tag="mm")
                        for kt in range(H // P):
                            nc.tensor.matmul(ps[:], w1h[oc // 8][:, kt, ts(oc % 8, P)],
                                             x_fm_bf[:, kt],
                                             start=(kt == 0), stop=(kt == H // P - 1))
                        nc.scalar.activation(z1[:, oc], ps[:], AF.Relu,
                                             bias=c1_sb[:, e, oc:oc + 1])
                    # gate = sigmoid(z1 @ gw2.T + gb2); h = gate*(x-ap)+ap  [H, T]
                    g2h = []
                    for hf in range(2):
                        w = wp.tile([P, I // P, H // 2], EXPERT_DT, tag="w")
                        nc.sync.dma_start(w[:], G2_d[e][:, :, ts(hf, H // 2)])
                        g2h.append(w)
                    hb = hp.tile([P, H // P, T], EXPERT_DT, tag="h")
                    for oc in range(H // P):
                        ps = ps_mm.tile([P, T], F32, tag="mm")
                        for kt in range(I // P):
                            nc.tensor.matmul(ps[:], g2h[oc // 4][:, kt, ts(oc % 4, P)],
                                             z1[:, kt],
                                             start=(kt == 0), stop=(kt == I // P - 1))
                        gt = gp.tile([P, T], F32, tag="gt")
                        nc.scalar.activation(gt[:], ps[:], AF.Sigmoid,
                                             bias=c2_sb[:, e, oc:oc + 1])
                        xm = gp.tile([P, T], F32, tag="xm")
                        nc.vector.tensor_single_scalar(xm[:], x_fm_bf[:, oc],
                                                       ap_sb[:, e, oc:oc + 1], ALU.subtract)
                        nc.vector.tensor_mul(xm[:], xm[:], gt[:])
                        nc.vector.tensor_single_scalar(hb[:, oc], xm[:],
                                                       ap_sb[:, e, oc:oc + 1], ALU.add)
                    # z2 = relu(h @ fw1.T + fb1)   [I, T]
                    f1h = []
                    for hf in range(2):
                        w = wp.tile([P, H // P, I // 2], EXPERT_DT, tag="w")
                        nc.sync.dma_start(w[:], F1_d[e][:, :, ts(hf, I // 2)])
                        f1h.append(w)
                    z2 = zp.tile([P, I // P, T], EXPERT_DT, tag="z")
                    for oc in range(I // P):
                        ps = ps_mm.tile([P, T], F32, tag="mm")
                        for kt in range(H // P):
                            nc.tensor.matmul(ps[:], f1h[oc // 8][:, kt, ts(oc % 8, P)],
                                             hb[:, kt],
                                             start=(kt == 0), stop=(kt == H // P - 1))
                        nc.scalar.activation(z2[:, oc], ps[:], AF.Relu,
                                             bias=b1_sb[:, e, oc:oc + 1])
                    # o = z2 @ fw2.T + fb2 (token-major out), r = o + x,
                    # y = LN(r)*g+b, out += prob_e * y
                    f2h = []
                    for hf in range(2):
                        w = wp.tile([P, I // P, H // 2], EXPERT_DT, tag="w")
                        nc.sync.dma_start(w[:], F2_d[e][:, :, ts(hf, H // 2)])
                        f2h.append(w)
                    fb2_sb = lnp.tile([P, H], F32, tag="fb2")
                    nc.sync.dma_start(fb2_sb[:], FB2_d[e])
                    if not ln_trivial:
                        lg_sb = lnp.tile([P, H], F32, tag="lg")
                        nc.sync.dma_start(lg_sb[:], LG_d[e])
                        lb_sb = lnp.tile([P, H], F32, tag="lb")
                        nc.sync.dma_start(lb_sb[:], LB_d[e])
                    for tcx in range(TCH):
                        t1 = ept.tile([P, H], F32, tag="t")
                        for hh in range(H // 512):
                            pso = ps_out.tile([P, 512], F32, tag="out")
                            for kt in range(I // P):
                                nc.tensor.matmul(pso[:],
                                                 z2[:, kt, ts(tcx, P)],
                                                 f2h[hh][:, kt],
                                                 start=(kt == 0), stop=(kt == I // P - 1))
                            nc.vector.tensor_add(t1[:, ts(hh, 512)], pso[:],
                                                 fb2_sb[:, ts(hh, 512)])
                        r = ept.tile([P, H], F32, tag="t")
                        s1 = sp.tile([P, 1], F32, tag="sp")
                        nc.vector.tensor_add(r[:], t1[:], x_tok[:, tcx])
                        nc.vector.reduce_sum(s1[:], r[:], axis=AXX)
                        s2 = sp.tile([P, 1], F32, tag="sp")
                        nc.scalar.activation(t1[:], r[:], AF.Square, accum_out=s2[:])
                        mu = sp.tile([P, 1], F32, tag="sp")
                        nc.vector.tensor_scalar_mul(mu[:], s1[:], 1.0 / H)
                        nm = sp.tile([P, 1], F32, tag="sp")
                        nc.vector.tensor_mul(nm[:], mu[:], mu[:])
                        nc.vector.tensor_scalar(nm[:], nm[:], -1.0, 1e-5, ALU.mult, ALU.add)
                        std = sp.tile([P, 1], F32, tag="sp")
                        nc.scalar.activation(std[:], s2[:], AF.Sqrt, bias=nm[:],
                                             scale=1.0 / H)
                        rstd = sp.tile([P, 1], F32, tag="sp")
                        nc.vector.reciprocal(rstd[:], std[:])
                        yn = ept.tile([P, H], F32, tag="t")
                        nc.vector.tensor_scalar(yn[:], r[:], mu[:], rstd[:],
                                                ALU.subtract, ALU.mult)
                        if not ln_trivial:
                            nc.vector.tensor_mul(yn[:], yn[:], lg_sb[:])
                            nc.vector.tensor_add(yn[:], yn[:], lb_sb[:])
                        wy = ept.tile([P, H], F32, tag="t")
                        nc.vector.tensor_single_scalar(wy[:], yn[:],
                                                       probs[:, tcx, e:e + 1], ALU.mult)
                        if e == 0:
                            nc.sync.dma_start(out_d[:, tcx], wy[:])
                        else:
                            nc.gpsimd.dma_start(out_d[:, tcx], wy[:], accum_op=ALU.add)

    nc.compile()
    return nc


def _pack_weights(i):
    f32 = np.float32
    gw1 = np.asarray(i["gate_w1"], f32)
    gb1 = np.asarray(i["gate_b1"], f32)
    gw2 = np.asarray(i["gate_w2"], f32)
    gb2 = np.asarray(i["gate_b2"], f32)
    fw1 = np.asarray(i["fc_w1"], f32)
    fb1 = np.asarray(i["fc_b1"], f32)
    fw2 = np.asarray(i["fc_w2"], f32)
    fb2 = np.asarray(i["fc_b2"], f32)
    ln_g = np.asarray(i["ln_g"], f32)
    ln_b = np.asarray(i["ln_b"], f32)
    ae = np.asarray(i["attr_emb"], f32)
    apw = np.asarray(i["attr_proj_w"], f32)
    apb = np.asarray(i["attr_proj_b"], f32)
    r_attr = np.asarray(i["r_attr"], f32)
    r_in_w = np.asarray(i["r_in_w"], f32)
    r_in_b = np.asarray(i["r_in_b"], f32)
    r_mid_w = np.asarray(i["r_mid_w"], f32)
    r_mid_b = np.asarray(i["r_mid_b"], f32)
    r_w = np.asarray(i["r_w"], f32)
    r_b = np.asarray(i["r_b"], f32)

    C = np.ascontiguousarray
    # input-independent precomputation
    ap_vec = np.einsum("ea,eha->eh", ae[:, 0], apw) + apb           # [E,H]
    c1 = np.einsum("eh,eih->ei", ap_vec, gw1[:, :, H:]) + gb1       # [E,I]
    W1x = gw1[:, :, :H]                                             # [E,I,H]

    def pack_lhsT(wT):  # [K,O] -> [P, K//P, O]
        K, O = wT.shape
        return C(wT.reshape(K // P, P, O).transpose(1, 0, 2))

    def chunks(v, nch):  # [E, nch*P] -> [P, E, nch]
        return C(np.stack([v[e].reshape(nch, P).T for e in range(E)], axis=1))

    d = {}
    r_in_wT = C(r_in_w.T)     # [1280, 4096]
    d["R1"] = C(r_in_wT.reshape(10, P, 32, P).transpose(1, 2, 0, 3))
    d["rib"] = C(r_in_b.reshape(32, P).T)
    r_mid_wT = C(r_mid_w.T)   # [4096, 1024]
    d["R2"] = C(r_mid_wT.reshape(32, P, 8, P).transpose(1, 2, 0, 3))
    d["rmb"] = C(r_mid_b.reshape(8, P).T)
    d["RW"] = C(r_w.T.reshape(8, P, E).transpose(1, 0, 2))
    d["rbb"] = C(np.tile(r_b, (P, 1)))
    d["RA"] = C(r_attr)
    d["W1"] = np.stack([pack_lhsT(C(W1x[e].T)) for e in range(E)]).astype(EXPERT_NP)
    d["C1"] = chunks(c1, I // P)
    d["G2"] = np.stack([pack_lhsT(C(gw2[e].T)) for e in range(E)]).astype(EXPERT_NP)
    d["C2"] = chunks(gb2, H // P)
    d["F1"] = np.stack([pack_lhsT(C(fw1[e].T)) for e in range(E)]).astype(EXPERT_NP)
    d["B1"] = chunks(fb1, I // P)
    d["F2"] = np.stack([pack_lhsT(C(fw2[e].T)) for e in range(E)]).astype(EXPERT_NP)
    d["FB2"] = C(np.tile(fb2[:, None, :], (1, P, 1)))
    d["LG"] = C(np.tile(ln_g[:, None, :], (1, P, 1)))
    d["LB"] = C(np.tile(ln_b[:, None, :], (1, P, 1)))
    d["APc"] = chunks(ap_vec, H // P)
    return d


def _pack_core_inputs(x, te, c):
    C = np.ascontiguousarray
    sl = slice(c * T, (c + 1) * T)
    xs = x.reshape(NTOK, H)[sl]                    # [T,H]
    te_s = te.reshape(NTOK, NT, TD)[sl]            # [T,NT,TD]
    xT = C(xs.T)                                   # [H,T]
    x_fm = C(xT.reshape(H // P, P, T).transpose(1, 0, 2))
    d = {
        "x_fm": x_fm,
        "x_fm_bf": x_fm.astype(EXPERT_NP),
        "x_tok": C(xs.reshape(TCH, P, H).transpose(1, 0, 2)),
        "te_rhs": C(te_s.reshape(T, A).T.reshape(A // P, P, T).transpose(1, 0, 2)),
        "te_attr": C(te_s.transpose(2, 1, 0)),
    }
    return d


def run_full(inputs, trace=False):
    global LAST_RESULTS
    x = np.asarray(inputs["x"], np.float32)
    te = np.asarray(inputs["task_embeddings"], np.float32)

    ln_trivial = bool(
        np.all(np.asarray(inputs["ln_g"], np.float32) == 1.0)
        and np.all(np.asarray(inputs["ln_b"], np.float32) == 0.0))
    key = ("prog", ln_trivial)
    if key not in _PROG_CACHE:
        _PROG_CACHE[key] = _build_program(ln_trivial=ln_trivial)
    nc = _PROG_CACHE[key]

    wmap = _pack_weights(inputs)
    in_maps = []
    for c in range(NCORE):
        m = dict(wmap)
        m.update(_pack_core_inputs(x, te, c))
        in_maps.append(m)

    res = run_bass_kernel_spmd(nc, in_maps, core_ids=list(range(NCORE)), trace=trace)
    LAST_RESULTS = res

    out = np.empty((NTOK, H), np.float32)
    ent_total = np.float64(0.0)
    for c in range(NCORE):
        oc = res.results[c]["out"]                 # [P, TCH, H]
        out[c * T:(c + 1) * T] = oc.transpose(1, 0, 2).reshape(T, H)
        ent_total += np.float64(res.results[c]["ent"].sum(dtype=np.float64))
    loss = np.float32(-(ent_total / NTOK))
    return out.reshape(B, S, H), loss


def kernel(**inputs):
    return run_full(inputs, trace=False)


# revision 17
# speedup vs baseline: 1.4286x; 1.0297x over previous
# MoE layer (8 experts, top-2 routing) on 8 trn2 NeuronCores.
# Sharding: token-parallel. Each core processes 512 of the 4096 tokens:
# full router + all 8 experts (dense eval, matching the reference) on its
# token slice. No collectives; host concatenates the per-core outputs.
#
# Layouts: activations are feature-major [feat_part, tok_free] through the
# MLPs (contraction dim always on partitions -> no transposes); the last
# expert matmul (fc2) swaps lhsT/rhs so its output lands token-major
# [tok_part, feat_free], where layernorm + the prob-weighted combine only
# need per-partition scalars. The input-independent attr projection and the
# ap-half of gate_w1 are folded into host-precomputed biases.
import numpy as np
import ml_dtypes
from contextlib import ExitStack

import concourse.bass as bass
import concourse.bacc as bacc
import concourse.mybir as mybir
import concourse.tile as tile
from concourse.bass import ts
from concourse.bass_utils import run_bass_kernel_spmd

F32 = mybir.dt.float32
F32R = mybir.dt.float32r
BF16 = mybir.dt.bfloat16
ROUTER_F32R = True   # run the two big router matmul layers as float32r
RDT = F32R if ROUTER_F32R else F32
AF = mybir.ActivationFunctionType
ALU = mybir.AluOpType
AXX = mybir.AxisListType.X

H, E, TD, NT, TOPK, I = 1024, 8, 64, 4, 2, 2048
A = TD * NT
B, S = 4, 1024
NTOK = B * S
NCORE = 8
T = NTOK // NCORE          # tokens per core
P = 128
TCH = T // P               # token chunks per core
EXPERT_NP = ml_dtypes.bfloat16
EXPERT_DT = BF16

_PROG_CACHE = {}
LAST_RESULTS = None


def _build_program(ln_trivial=False):
    nc = bacc.Bacc("TRN2", target_bir_lowering=False, debug=False,
                   enable_asserts=False, num_devices=NCORE)

    def din(name, shape, dt=F32):
        return nc.dram_tensor(name, list(shape), dt, kind="ExternalInput").ap()

    # per-core inputs
    x_fm_bf_d = din("x_fm_bf", [P, H // P, T], EXPERT_DT)
    x_fm_d = din("x_fm", [P, H // P, T], RDT)
    x_tok_d = din("x_tok", [P, TCH, H])
    te_rhs_d = din("te_rhs", [P, A // P, T], RDT)
    te_attr_d = din("te_attr", [TD, NT, T])
    # replicated router weights
    R1_d = din("R1", [P, 4 * H // P, 10, P], RDT)       # r_in_w^T packed (ki, oc, ko, oi)
    rib_d = din("rib", [P, 4 * H // P])
    R2_d = din("R2", [P, H // P, 32, P], RDT)
    rmb_d = din("rmb", [P, H // P])
    RW_d = din("RW", [P, H // P, E])
    rbb_d = din("rbb", [P, E])
    RA_d = din("RA", [TD, E])
    # replicated expert weights (stacked on E)
    W1_d = din("W1", [E, P, H // P, I], EXPERT_DT)
    C1_d = din("C1", [P, E, I // P])
    G2_d = din("G2", [E, P, I // P, H], EXPERT_DT)
    C2_d = din("C2", [P, E, H // P])
    F1_d = din("F1", [E, P, H // P, I], EXPERT_DT)
    B1_d = din("B1", [P, E, I // P])
    F2_d = din("F2", [E, P, I // P, H], EXPERT_DT)
    FB2_d = din("FB2", [E, P, H])
    LG_d = din("LG", [E, P, H])
    LB_d = din("LB", [E, P, H])
    AP_d = din("APc", [P, E, H // P])

    out_d = nc.dram_tensor("out", [E, P, TCH, H], F32, kind="ExternalOutput").ap()
    ent_d = nc.dram_tensor("ent", [P, TCH], F32, kind="ExternalOutput").ap()

    with tile.TileContext(nc) as tc:
        with ExitStack() as top:
            const = top.enter_context(tc.tile_pool(name="const", bufs=1))
            ps_mm = top.enter_context(tc.tile_pool(name="ps_mm", bufs=2, space="PSUM"))

            x_fm_bf = const.tile([P, H // P, T], EXPERT_DT)
            for i in range(H // P):
                nc.sync.dma_start(x_fm_bf[:, i], x_fm_bf_d[:, i])
            x_tok = const.tile([P, TCH, H], F32)
            for i in range(TCH):
                nc.sync.dma_start(x_tok[:, i], x_tok_d[:, i])
            c1_sb = const.tile([P, E, I // P], F32)
            nc.sync.dma_start(c1_sb[:], C1_d)
            c2_sb = const.tile([P, E, H // P], F32)
            nc.sync.dma_start(c2_sb[:], C2_d)
            b1_sb = const.tile([P, E, I // P], F32)
            nc.sync.dma_start(b1_sb[:], B1_d)
            ap_sb = const.tile([P, E, H // P], F32)
            nc.sync.dma_start(ap_sb[:], AP_d)
            probs = const.tile([P, TCH, E], F32)
            ent = const.tile([P, TCH], F32)
            eps8 = const.tile([P, 1], F32)
            nc.vector.memset(eps8[:], 1e-8)

            # ---------------- router (fp32) ----------------
            with ExitStack() as rs:
                rpool = rs.enter_context(tc.tile_pool(name="rconst", bufs=1))
                r1p = rs.enter_context(tc.tile_pool(name="r1", bufs=4))
                r2p = rs.enter_context(tc.tile_pool(name="r2", bufs=2))
                h1p = rs.enter_context(tc.tile_pool(name="h1", bufs=1))
                ps_sm = rs.enter_context(tc.tile_pool(name="ps_sm", bufs=2, space="PSUM"))
                smt = rs.enter_context(tc.tile_pool(name="smt", bufs=4))

                x_fm = rpool.tile([P, H // P, T], RDT)
                for i in range(H // P):
                    nc.sync.dma_start(x_fm[:, i], x_fm_d[:, i])
                te_rhs = rpool.tile([P, A // P, T], RDT)
                for i in range(A // P):
                    nc.sync.dma_start(te_rhs[:, i], te_rhs_d[:, i])
                te_attr = rpool.tile([TD, NT, T], F32)
                nc.sync.dma_start(te_attr[:], te_attr_d)
                ra_sb = rpool.tile([TD, E], F32)
                nc.sync.dma_start(ra_sb[:], RA_d)
                rib_sb = rpool.tile([P, 4 * H // P], F32)
                nc.sync.dma_start(rib_sb[:], rib_d)
                rmb_sb = rpool.tile([P, H // P], F32)
                nc.sync.dma_start(rmb_sb[:], rmb_d)
                rw_sb = rpool.tile([P, H // P, E], F32)
                nc.sync.dma_start(rw_sb[:], RW_d)
                rbb_sb = rpool.tile([P, E], F32)
                nc.sync.dma_start(rbb_sb[:], rbb_d)

                h1 = h1p.tile([P, 4 * H // P, T], RDT)
                h2 = rpool.tile([P, H // P, T], F32)

                rhs_list = [x_fm[:, i] for i in range(H // P)] + \
                           [te_rhs[:, i] for i in range(A // P)]
                # iv1 = relu([x, te] @ r_in_w.T + r_in_b)   [4H, T]
                for oc in range(4 * H // P):
                    w = r1p.tile([P, 10, P], RDT, tag="r1w")
                    nc.sync.dma_start(w[:], R1_d[:, oc])
                    ps = ps_mm.tile([P, T], F32, tag="mm")
                    for kt in range(10):
                        nc.tensor.matmul(ps[:], w[:, kt], rhs_list[kt],
                                         start=(kt == 0), stop=(kt == 9))
                    nc.scalar.activation(h1[:, oc], ps[:], AF.Relu,
                                         bias=rib_sb[:, oc:oc + 1])
                # iv2 = relu(iv1 @ r_mid_w.T + r_mid_b)   [H, T]
                for oc in range(H // P):
                    w = r2p.tile([P, 32, P], RDT, tag="r2w")
                    nc.sync.dma_start(w[:], R2_d[:, oc])
                    ps = ps_mm.tile([P, T], F32, tag="mm")
                    for kt in range(32):
                        nc.tensor.matmul(ps[:], w[:, kt], h1[:, kt],
                                         start=(kt == 0), stop=(kt == 31))
                    nc.scalar.activation(h2[:, oc], ps[:], AF.Relu,
                                         bias=rmb_sb[:, oc:oc + 1])
                # logits / softmax / attr softmax / top-2 per token chunk
                for tcx in range(TCH):
                    ps = ps_sm.tile([P, E], F32, tag="sm")
                    for kt in range(H // P):
                        nc.tensor.matmul(ps[:], h2[:, kt, ts(tcx, P)], rw_sb[:, kt],
                                         start=(kt == 0), stop=(kt == H // P - 1))
                    lg = smt.tile([P, E], F32, tag="s8")
                    nc.vector.tensor_add(lg[:], ps[:], rbb_sb[:])
                    nmax = smt.tile([P, 1], F32, tag="s1")
                    nc.vector.reduce_max(nmax[:], lg[:], axis=AXX, negate=True)
                    sume = smt.tile([P, 1], F32, tag="s1")
                    ep = smt.tile([P, E], F32, tag="s8")
                    nc.scalar.activation(ep[:], lg[:], AF.Exp, bias=nmax[:],
                                         accum_out=sume[:])
                    rec = smt.tile([P, 1], F32, tag="s1")
                    nc.vector.reciprocal(rec[:], sume[:])
                    nc.vector.tensor_scalar_mul(ep[:], ep[:], rec[:])
                    asum = smt.tile([P, E], F32, tag="s8")
                    for n in range(NT):
                        psa = ps_sm.tile([P, E], F32, tag="sm")
                        nc.tensor.matmul(psa[:], te_attr[:, n, ts(tcx, P)], ra_sb[:],
                                         start=True, stop=True)
                        nma = smt.tile([P, 1], F32, tag="s1")
                        nc.vector.reduce_max(nma[:], psa[:], axis=AXX, negate=True)
                        suma = smt.tile([P, 1], F32, tag="s1")
                        an = smt.tile([P, E], F32, tag="s8")
                        nc.scalar.activation(an[:], psa[:], AF.Exp, bias=nma[:],
                                             accum_out=suma[:])
                        reca = smt.tile([P, 1], F32, tag="s1")
                        nc.vector.reciprocal(reca[:], suma[:])
                        if n == 0:
                            nc.vector.tensor_scalar_mul(asum[:], an[:], reca[:])
                        else:
                            tmp8 = smt.tile([P, E], F32, tag="s8")
                            nc.vector.tensor_scalar_mul(tmp8[:], an[:], reca[:])
                            nc.vector.tensor_add(asum[:], asum[:], tmp8[:])
                    pf = smt.tile([P, E], F32, tag="s8")
                    nc.vector.tensor_mul(pf[:], ep[:], asum[:])
                    nc.vector.tensor_scalar_mul(pf[:], pf[:], 1.0 / NT)
                    # top-2 mask (values >= 2nd max)
                    m1 = smt.tile([P, 1], F32, tag="s1")
                    nc.vector.reduce_max(m1[:], pf[:], axis=AXX)
                    eq = smt.tile([P, E], F32, tag="s8")
                    nc.vector.tensor_single_scalar(eq[:], pf[:], m1[:], ALU.is_ge)
                    nc.vector.tensor_scalar_mul(eq[:], eq[:], -1e30)
                    p2t = smt.tile([P, E], F32, tag="s8")
                    nc.vector.tensor_add(p2t[:], pf[:], eq[:])
                    m2 = smt.tile([P, 1], F32, tag="s1")
                    nc.vector.reduce_max(m2[:], p2t[:], axis=AXX)
                    msk = smt.tile([P, E], F32, tag="s8")
                    nc.vector.tensor_single_scalar(msk[:], pf[:], m2[:], ALU.is_ge)
                    nc.vector.tensor_mul(probs[:, tcx], pf[:], msk[:])
                    # entropy partial: sum_e p*log(p+1e-8)
                    l8 = smt.tile([P, E], F32, tag="s8")
                    nc.scalar.activation(l8[:], probs[:, tcx], AF.Ln, bias=eps8[:])
                    scr = smt.tile([P, E], F32, tag="s8")
                    nc.vector.tensor_mul(scr[:], probs[:, tcx], l8[:])
                    nc.vector.reduce_sum(ent[:, tcx:tcx + 1], scr[:], axis=AXX)
                nc.sync.dma_start(ent_d, ent[:])

            # ---------------- experts (bf16) ----------------
            with ExitStack() as es:
                wp = es.enter_context(tc.tile_pool(name="w", bufs=6 if ln_trivial else 5))
                zp = es.enter_context(tc.tile_pool(name="z", bufs=2))
                gp = es.enter_context(tc.tile_pool(name="g", bufs=3))
                hp = es.enter_context(tc.tile_pool(name="h", bufs=2))
                lnp = es.enter_context(tc.tile_pool(name="ln", bufs=2))
                ept = es.enter_context(tc.tile_pool(name="ept", bufs=3))
                sp = es.enter_context(tc.tile_pool(name="sp", bufs=6))
                ps_out = es.enter_context(tc.tile_pool(name="ps_out", bufs=4, space="PSUM"))

                for e in range(E):
                    # z1 = relu(x @ W1x.T + c1)    [I, T]
                    w1h = []
                    for hf in range(2):
                        w = wp.tile([P, H // P, I // 2], EXPERT_DT, tag="w")
                        nc.sync.dma_start(w[:], W1_d[e][:, :, ts(hf, I // 2)])
                        w1h.append(w)
                    z1 = zp.tile([P, I // P, T], EXPERT_DT, tag="z")
                    for oc in range(I // P):
                        ps = ps_mm.tile([P, T], F32, tag="mm")
                        for kt in range(H // P):
                            nc.tensor.matmul(ps[:], w1h[oc // 8][:, kt, ts(oc % 8, P)],
                                             x_fm_bf[:, kt],
                                             start=(kt == 0), stop=(kt == H // P - 1))
                        nc.scalar.activation(z1[:, oc], ps[:], AF.Relu,
                                             bias=c1_sb[:, e, oc:oc + 1])
                    # gate = sigmoid(z1 @ gw2.T + gb2); h = gate*(x-ap)+ap  [H, T]
                    g2h = []
                    for hf in range(2):
                        w = wp.tile([P, I // P, H // 2], EXPERT_DT, tag="w")
                        nc.sync.dma_start(w[:], G2_d[e][:, :, ts(hf, H // 2)])
                        g2h.append(w)
                    hb = hp.tile([P, H // P, T], EXPERT_DT, tag="h")
                    for oc in range(H // P):
                        ps = ps_mm.tile([P, T], F32, tag="mm")
                        for kt in range(I // P):
                            nc.tensor.matmul(ps[:], g2h[oc // 4][:, kt, ts(oc % 4, P)],
                                             z1[:, kt],
                                             start=(kt == 0), stop=(kt == I // P - 1))
                        gt = gp.tile([P, T], F32, tag="gt")
                        nc.scalar.activation(gt[:], ps[:], AF.Sigmoid,
                                             bias=c2_sb[:, e, oc:oc + 1])
                        xm = gp.tile([P, T], F32, tag="xm")
                        nc.vector.tensor_single_scalar(xm[:], x_fm_bf[:, oc],
                                                       ap_sb[:, e, oc:oc + 1], ALU.subtract)
                        nc.vector.tensor_mul(xm[:], xm[:], gt[:])
                        nc.vector.tensor_single_scalar(hb[:, oc], xm[:],
                                                       ap_sb[:, e, oc:oc + 1], ALU.add)
                    # z2 = relu(h @ fw1.T + fb1)   [I, T]
                    f1h = []
                    for hf in range(2):
                        w = wp.tile([P, H // P, I // 2], EXPERT_DT, tag="w")
                        nc.sync.dma_start(w[:], F1_d[e][:, :, ts(hf, I // 2)])
                        f1h.append(w)
                    z2 = zp.tile([P, I // P, T], EXPERT_DT, tag="z")
                    for oc in range(I // P):
                        ps = ps_mm.tile([P, T], F32, tag="mm")
                        for kt in range(H // P):
                            nc.tensor.matmul(ps[:], f1h[oc // 8][:, kt, ts(oc % 8, P)],
                                             hb[:, kt],
                                             start=(kt == 0), stop=(kt == H // P - 1))
                        nc.scalar.activation(z2[:, oc], ps[:], AF.Relu,
                                             bias=b1_sb[:, e, oc:oc + 1])
                    # o = z2 @ fw2.T + fb2 (token-major out), r = o + x,
                    # y = LN(r)*g+b, out += prob_e * y
                    f2h = []
                    for hf in range(2):
                        w = wp.tile([P, I // P, H // 2], EXPERT_DT, tag="w")
                        nc.sync.dma_start(w[:], F2_d[e][:, :, ts(hf, H // 2)])
                        f2h.append(w)
                    fb2_sb = lnp.tile([P, H], F32, tag="fb2")
                    nc.sync.dma_start(fb2_sb[:], FB2_d[e])
                    if not ln_trivial:
                        lg_sb = lnp.tile([P, H], F32, tag="lg")
                        nc.sync.dma_start(lg_sb[:], LG_d[e])
                        lb_sb = lnp.tile([P, H], F32, tag="lb")
                        nc.sync.dma_start(lb_sb[:], LB_d[e])
                    for tcx in range(TCH):
                        t1 = ept.tile([P, H], F32, tag="t")
                        for hh in range(H // 512):
                            pso = ps_out.tile([P, 512], F32, tag="out")
                            for kt in range(I // P):
                                nc.tensor.matmul(pso[:],
                                                 z2[:, kt, ts(tcx, P)],
                                                 f2h[hh][:, kt],
                                                 start=(kt == 0), stop=(kt == I // P - 1))
                            nc.vector.tensor_add(t1[:, ts(hh, 512)], pso[:],
                                                 fb2_sb[:, ts(hh, 512)])
                        r = ept.tile([P, H], F32, tag="t")
                        s1 = sp.tile([P, 1], F32, tag="sp")
                        nc.vector.tensor_add(r[:], t1[:], x_tok[:, tcx])
                        nc.vector.reduce_sum(s1[:], r[:], axis=AXX)
                        s2 = sp.tile([P, 1], F32, tag="sp")
                        nc.scalar.activation(t1[:], r[:], AF.Square, accum_out=s2[:])
                        mu = sp.tile([P, 1], F32, tag="sp")
                        nc.vector.tensor_scalar_mul(mu[:], s1[:], 1.0 / H)
                        nm = sp.tile([P, 1], F32, tag="sp")
                        nc.vector.tensor_mul(nm[:], mu[:], mu[:])
                        nc.vector.tensor_scalar(nm[:], nm[:], -1.0, 1e-5, ALU.mult, ALU.add)
                        std = sp.tile([P, 1], F32, tag="sp")
                        nc.scalar.activation(std[:], s2[:], AF.Sqrt, bias=nm[:],
                                             scale=1.0 / H)
                        rstd = sp.tile([P, 1], F32, tag="sp")
                        nc.vector.reciprocal(rstd[:], std[:])
                        yn = ept.tile([P, H], F32, tag="t")
                        nc.vector.tensor_scalar(yn[:], r[:], mu[:], rstd[:],
                                                ALU.subtract, ALU.mult)
                        if not ln_trivial:
                            nc.vector.tensor_mul(yn[:], yn[:], lg_sb[:])
                            nc.vector.tensor_add(yn[:], yn[:], lb_sb[:])
                        wy = ept.tile([P, H], F32, tag="t")
                        nc.vector.tensor_single_scalar(wy[:], yn[:],
                                                       probs[:, tcx, e:e + 1], ALU.mult)
                        nc.sync.dma_start(out_d[e, :, tcx], wy[:])

    nc.compile()
    return nc


def _pack_weights(i):
    f32 = np.float32
    gw1 = np.asarray(i["gate_w1"], f32)
    gb1 = np.asarray(i["gate_b1"], f32)
    gw2 = np.asarray(i["gate_w2"], f32)
    gb2 = np.asarray(i["gate_b2"], f32)
    fw1 = np.asarray(i["fc_w1"], f32)
    fb1 = np.asarray(i["fc_b1"], f32)
    fw2 = np.asarray(i["fc_w2"], f32)
    fb2 = np.asarray(i["fc_b2"], f32)
    ln_g = np.asarray(i["ln_g"], f32)
    ln_b = np.asarray(i["ln_b"], f32)
    ae = np.asarray(i["attr_emb"], f32)
    apw = np.asarray(i["attr_proj_w"], f32)
    apb = np.asarray(i["attr_proj_b"], f32)
    r_attr = np.asarray(i["r_attr"], f32)
    r_in_w = np.asarray(i["r_in_w"], f32)
    r_in_b = np.asarray(i["r_in_b"], f32)
    r_mid_w = np.asarray(i["r_mid_w"], f32)
    r_mid_b = np.asarray(i["r_mid_b"], f32)
    r_w = np.asarray(i["r_w"], f32)
    r_b = np.asarray(i["r_b"], f32)

    C = np.ascontiguousarray
    # input-independent precomputation
    ap_vec = np.einsum("ea,eha->eh", ae[:, 0], apw) + apb           # [E,H]
    c1 = np.einsum("eh,eih->ei", ap_vec, gw1[:, :, H:]) + gb1       # [E,I]
    W1x = gw1[:, :, :H]                                             # [E,I,H]

    def pack_lhsT(wT):  # [K,O] -> [P, K//P, O]
        K, O = wT.shape
        return C(wT.reshape(K // P, P, O).transpose(1, 0, 2))

    def chunks(v, nch):  # [E, nch*P] -> [P, E, nch]
        return C(np.stack([v[e].reshape(nch, P).T for e in range(E)], axis=1))

    d = {}
    r_in_wT = C(r_in_w.T)     # [1280, 4096]
    d["R1"] = C(r_in_wT.reshape(10, P, 32, P).transpose(1, 2, 0, 3))
    d["rib"] = C(r_in_b.reshape(32, P).T)
    r_mid_wT = C(r_mid_w.T)   # [4096, 1024]
    d["R2"] = C(r_mid_wT.reshape(32, P, 8, P).transpose(1, 2, 0, 3))
    d["rmb"] = C(r_mid_b.reshape(8, P).T)
    d["RW"] = C(r_w.T.reshape(8, P, E).transpose(1, 0, 2))
    d["rbb"] = C(np.tile(r_b, (P, 1)))
    d["RA"] = C(r_attr)
    d["W1"] = np.stack([pack_lhsT(C(W1x[e].T)) for e in range(E)]).astype(EXPERT_NP)
    d["C1"] = chunks(c1, I // P)
    d["G2"] = np.stack([pack_lhsT(C(gw2[e].T)) for e in range(E)]).astype(EXPERT_NP)
    d["C2"] = chunks(gb2, H // P)
    d["F1"] = np.stack([pack_lhsT(C(fw1[e].T)) for e in range(E)]).astype(EXPERT_NP)
    d["B1"] = chunks(fb1, I // P)
    d["F2"] = np.stack([pack_lhsT(C(fw2[e].T)) for e in range(E)]).astype(EXPERT_NP)
    d["FB2"] = C(np.tile(fb2[:, None, :], (1, P, 1)))
    d["LG"] = C(np.tile(ln_g[:, None, :], (1, P, 1)))
    d["LB"] = C(np.tile(ln_b[:, None, :], (1, P, 1)))
    d["APc"] = chunks(ap_vec, H // P)
    return d


def _pack_core_inputs(x, te, c):
    C = np.ascontiguousarray
    sl = slice(c * T, (c + 1) * T)
    xs = x.reshape(NTOK, H)[sl]                    # [T,H]
    te_s = te.reshape(NTOK, NT, TD)[sl]            # [T,NT,TD]
    xT = C(xs.T)                                   # [H,T]
    x_fm = C(xT.reshape(H // P, P, T).transpose(1, 0, 2))
    d = {
        "x_fm": x_fm,
        "x_fm_bf": x_fm.astype(EXPERT_NP),
        "x_tok": C(xs.reshape(TCH, P, H).transpose(1, 0, 2)),
        "te_rhs": C(te_s.reshape(T, A).T.reshape(A // P, P, T).transpose(1, 0, 2)),
        "te_attr": C(te_s.transpose(2, 1, 0)),
    }
    return d


def run_full(inputs, trace=False):
    global LAST_RESULTS
    x = np.asarray(inputs["x"], np.float32)
    te = np.asarray(inputs["task_embeddings"], np.float32)

    ln_trivial = bool(
        np.all(np.asarray(inputs["ln_g"], np.float32) == 1.0)
        and np.all(np.asarray(inputs["ln_b"], np.float32) == 0.0))
    key = ("prog", ln_trivial)
    if key not in _PROG_CACHE:
        _PROG_CACHE[key] = _build_program(ln_trivial=ln_trivial)
    nc = _PROG_CACHE[key]

    wmap = _pack_weights(inputs)
    in_maps = []
    for c in range(NCORE):
        m = dict(wmap)
        m.update(_pack_core_inputs(x, te, c))
        in_maps.append(m)

    res = run_bass_kernel_spmd(nc, in_maps, core_ids=list(range(NCORE)), trace=trace)
    LAST_RESULTS = res

    out = np.empty((NTOK, H), np.float32)
    ent_total = np.float64(0.0)
    for c in range(NCORE):
        oc = res.results[c]["out"].sum(axis=0)     # [E,P,TCH,H] -> [P,TCH,H]
        out[c * T:(c + 1) * T] = oc.transpose(1, 0, 2).reshape(T, H)
        ent_total += np.float64(res.results[c]["ent"].sum(dtype=np.float64))
    loss = np.float32(-(ent_total / NTOK))
    return out.reshape(B, S, H), loss


def kernel(**inputs):
    return run_full(inputs, trace=False)
